# revision 1
# baseline (speedup 1.0000x reference)
"""Trainium2 Bass kernel for nn_CausalSelfAttention (B=4, L=2048, D=1024, H=16).

Sharding: 2 heads per core (tensor parallel) x 8 cores, all batches on every
core.  Each core computes qkv for its 2 heads over all tokens (reading full x),
runs causal attention, and produces a partial projection output
yT_c = proj_w[rows_c].T @ O_c^T  of shape [D, B*L].  The host sums the 8
partials, transposes, and adds proj_b.

Device pipeline per core (all matmuls fp32r: 1 cycle/row at N>=256):
  x [tok,d] --PE transpose--> xT tiles [d,tok]
  qkvT [feat,tok] = w_tile.T @ xT (+bias via K=1 matmul)
  RoPE: rot(q) via signed-permutation matmul on PE, then
        qT_roped = qT*cos + rot(qT)*sin  (3 full-height DVE ops)
  V: PE-transpose back to tok-major, stored as [V|1] tiles
  S^T[k,q] = KT_slice.T @ QT_slice (heads interleaved -> PE row-group overlap)
  P^T = exp(S^T/8) (ACT, causal via affine_select on diagonal stripes)
  O^T[hd+1,q] += [V|1].T @ P^T
  normalize via reciprocal_approx_fast + gpsimd partition_broadcast + DVE mul
  yT += pw_h.T @ OT_h (two K=64 matmuls per tile)
The normalize+projection of iteration i-1 is emitted after the qkv phase of
iteration i so the PE never idles on the normalization chain (HAM stays warm).
"""

import numpy as np

import concourse.bass as bass  # noqa: F401
import concourse.tile as tile
from concourse import mybir, bacc
from concourse import bass_utils
from concourse.masks import make_identity

f32 = mybir.dt.float32
f32r = mybir.dt.float32r
AL = mybir.AluOpType
AF = mybir.ActivationFunctionType


class _Bacc(bacc.Bacc):
    """Bacc that pins all activations to the one table set holding both
    ln and exp (plus copy/identity), so the per-iteration Ln<->Exp pair
    doesn't thrash ACT_TABLE_LOADs (~2.7us each)."""

    def insert_act_table_loads(self):
        import bass_rust as _bass_rust
        from concourse.hw_specs import get_activation_tables

        has_activation = any(
            isinstance(i, mybir.InstActivation)
            for bb in self.main_func.blocks
            for i in bb.instructions
        )
        if not has_activation:
            return
        # act_func_set_id is positional: keep the full list order, but empty
        # every other set so the chooser can only pick the combined one.
        tables = [
            (k, v if k == "natural_log_exp_and_others" else set())
            for k, v in get_activation_tables(self.m.arch).items()
        ]
        _bass_rust.insert_act_table_loads(self, tables)

HIDDEN = 1024
HEADS = 16
HD = 64
ROPE_BASE = 10000.0
N_CORES = 8
H2 = 2           # heads per core
F = 3 * H2 * HD  # 384 qkv feature columns per core
QCH = 512        # token chunk = attention q granule
DT = HIDDEN // 128  # 8 d tiles


def build_program(NB, T):
    """Build the per-core Bass program: NB batches of T tokens each."""
    assert T % QCH == 0
    NTOK = NB * T
    NKT = T // 128  # k tiles per batch
    nc = _Bacc("TRN2", target_bir_lowering=False, debug=False,
               num_devices=N_CORES)

    x = nc.dram_tensor("x", [NTOK, HIDDEN], f32r, kind="ExternalInput").ap()
    w = nc.dram_tensor("w", [HIDDEN, F], f32r, kind="ExternalInput").ap()
    brow = nc.dram_tensor("brow", [1, F], f32r, kind="ExternalInput").ap()
    psgn = nc.dram_tensor("psgn", [128, 128], f32r, kind="ExternalInput").ap()
    pw = nc.dram_tensor("pw", [128, HIDDEN], f32r, kind="ExternalInput").ap()
    cos_t = nc.dram_tensor("cos_t", [128, T], f32, kind="ExternalInput").ap()
    sin_t = nc.dram_tensor("sin_t", [128, T], f32, kind="ExternalInput").ap()
    yT = nc.dram_tensor("yT", [HIDDEN, NTOK], f32, kind="ExternalOutput").ap()

    with tile.TileContext(nc) as tc:
        with tc.tile_pool(name="const", bufs=1) as constp, \
             tc.tile_pool(name="resident", bufs=1) as resp, \
             tc.tile_pool(name="xload", bufs=6) as xp, \
             tc.tile_pool(name="xt", bufs=12) as xtp, \
             tc.tile_pool(name="rope", bufs=3) as ropep, \
             tc.tile_pool(name="qtcur", bufs=2) as qtp, \
             tc.tile_pool(name="pt", bufs=4) as ptp, \
             tc.tile_pool(name="ot", bufs=3) as otp, \
             tc.tile_pool(name="ysb", bufs=3) as yp, \
             tc.tile_pool(name="small", bufs=4) as smp, \
             tc.tile_pool(name="ps_s", bufs=2, space="PSUM") as ps_s_p, \
             tc.tile_pool(name="ps_o", bufs=2, space="PSUM") as ps_o_p, \
             tc.tile_pool(name="ps_m", bufs=2, space="PSUM") as ps_m_p:

            # ---- constants / residents ----
            ident_f = constp.tile([128, 128], f32)
            make_identity(nc, ident_f[:])
            ident = constp.tile([128, 128], f32r)
            nc.vector.tensor_copy(ident[:], ident_f[:])
            # w tiles: per d-tile, F columns
            w_sb = constp.tile([128, DT * F], f32r)
            for dt in range(DT):
                nc.sync.dma_start(w_sb[:, dt * F:(dt + 1) * F],
                                  w[dt * 128:(dt + 1) * 128, :])
            brow_sb = constp.tile([1, F], f32r)
            nc.sync.dma_start(brow_sb[:], brow[:])
            psgn_sb = constp.tile([128, 128], f32r)
            nc.sync.dma_start(psgn_sb[:], psgn[:])
            ones_f = constp.tile([128, 512], f32)
            nc.gpsimd.memset(ones_f[:], 1.0)
            ones_row = constp.tile([1, 512], f32r)
            nc.vector.tensor_copy(ones_row[:], ones_f[0:1, :])
            pw_sb = constp.tile([128, HIDDEN], f32r)
            nc.sync.dma_start(pw_sb[:], pw[:])
            cos_sb = constp.tile([128, T], f32)
            nc.sync.dma_start(cos_sb[:], cos_t[:])
            sin_sb = constp.tile([128, T], f32)
            nc.sync.dma_start(sin_sb[:], sin_t[:])

            KT_res = resp.tile([128, T], f32r)
            V_res = resp.tile([128, NKT * 130], f32r)
            v4 = V_res[:].rearrange("p (kt h c) -> p kt h c", kt=NKT, h=2)
            nc.gpsimd.tensor_copy(
                v4[:, :, :, 64],
                ones_f[:, :2 * NKT].rearrange("p (kt h) -> p kt h", kt=NKT))

            def norm_part(st):
                O, t0v = st
                ot_full = otp.tile([128, 512], f32r, tag="ot", name="ot_full")
                for h in range(2):
                    # 1/rowsum = exp(-ln(rowsum)) on ACT (same table set as
                    # the attention Exp; DVE reciprocal is 3.3us and would
                    # stall the pipeline)
                    lnv = smp.tile([1, 512], f32, tag="ln", name="lnv")
                    nc.scalar.activation(lnv[:], O[h][64:65, :], AF.Ln)
                    rs_sb = smp.tile([1, 512], f32, tag="rs", name="rs")
                    nc.scalar.activation(rs_sb[:], lnv[:], AF.Exp,
                                         bias=0.0, scale=-1.0)
                    rsb = smp.tile([64, 512], f32, tag="rsb", name="rsb")
                    nc.gpsimd.partition_broadcast(rsb[:], rs_sb[:])
                    nc.vector.tensor_tensor(ot_full[64 * h:64 * h + 64, :],
                                            O[h][0:64, :], rsb[:], AL.mult)
                return ot_full

            def proj_part(st, ot_full):
                O, t0v = st
                for oi in range(8):
                    ps_y = ps_m_p.tile([128, 512], f32, tag="m", name="ps_y")
                    nc.tensor.matmul(
                        ps_y[:], pw_sb[:, oi * 128:(oi + 1) * 128],
                        ot_full[:], start=True, stop=True)
                    ysb = yp.tile([128, 512], f32, tag="y", name="ysb")
                    if oi % 2 == 0:
                        nc.vector.tensor_copy(ysb[:], ps_y[:])
                    else:
                        nc.scalar.copy(ysb[:], ps_y[:])
                    nc.sync.dma_start(
                        yT[oi * 128:(oi + 1) * 128, t0v:t0v + 512], ysb[:])

            prev = None
            for b in range(NB):
                for qc in range(T // QCH):
                    Q0 = qc * QCH
                    t0 = b * T + Q0
                    prev_ots = norm_part(prev) if prev is not None else None
                    # ---------- qkv phase for tokens [t0, t0+512) ----------
                    xa = [xp.tile([128, HIDDEN], f32r, tag="x", name=f"xa{tt}")
                          for tt in range(4)]
                    for tt in range(4):
                        nc.sync.dma_start(
                            xa[tt][:], x[t0 + tt * 128: t0 + (tt + 1) * 128, :])
                    xt_sb = [xtp.tile([128, QCH], f32r, tag="xt",
                                      name=f"xt{dt}") for dt in range(DT)]
                    for dt in range(DT):
                        ps_xt = ps_m_p.tile([128, QCH], f32r, tag="m",
                                            name="ps_xt")
                        for tt in range(4):
                            nc.tensor.transpose(
                                ps_xt[:, tt * 128:(tt + 1) * 128],
                                xa[tt][:, dt * 128:(dt + 1) * 128], ident[:])
                        nc.vector.tensor_copy(xt_sb[dt][:], ps_xt[:])
                    QT_cur = qtp.tile([128, QCH], f32r, tag="qt", name="QT")
                    for f in range(3):  # 0=q, 1=k, 2=v
                        ps_f = ps_m_p.tile([128, QCH], f32, tag="m",
                                           name="ps_f")
                        for dt in range(DT):
                            nc.tensor.matmul(
                                ps_f[:],
                                w_sb[:, dt * F + f * 128:dt * F + (f + 1) * 128],
                                xt_sb[dt][:], start=(dt == 0), stop=False)
                        nc.tensor.matmul(
                            ps_f[:], brow_sb[:, f * 128:(f + 1) * 128],
                            ones_row[:], start=False, stop=True)
                        raw = ropep.tile([128, QCH], f32r, tag="raw",
                                         name="raw")
                        nc.scalar.copy(raw[:], ps_f[:])
                        if f < 2:
                            ps_rot = ps_m_p.tile([128, QCH], f32, tag="m",
                                                 name="ps_rot")
                            nc.tensor.matmul(ps_rot[:], psgn_sb[:], raw[:],
                                             start=True, stop=True)
                            t1 = ropep.tile([128, QCH], f32, tag="t1",
                                            name="t1")
                            nc.vector.tensor_tensor(
                                t1[:], raw[:], cos_sb[:, Q0:Q0 + QCH], AL.mult)
                            t2 = ropep.tile([128, QCH], f32, tag="t2",
                                            name="t2")
                            nc.vector.tensor_tensor(
                                t2[:], ps_rot[:], sin_sb[:, Q0:Q0 + QCH],
                                AL.mult)
                            dst = (QT_cur[:] if f == 0
                                   else KT_res[:, Q0:Q0 + QCH])
                            nc.vector.tensor_tensor(dst, t1[:], t2[:], AL.add)
                        else:
                            for tt in range(4):
                                ps_vt = ps_m_p.tile([128, 128], f32r, tag="m",
                                                    name="ps_vt")
                                nc.tensor.transpose(
                                    ps_vt[:],
                                    raw[:, tt * 128:(tt + 1) * 128], ident[:])
                                kt = Q0 // 128 + tt
                                nc.vector.tensor_copy(
                                    v4[:, kt, :, 0:64],
                                    ps_vt[:].rearrange("p (h j) -> p h j", h=2))
                    # ---------- deferred projection ----------
                    if prev is not None:
                        proj_part(prev, prev_ots)
                    # ---------- attention for (b, qc) ----------
                    nkb = (Q0 + QCH) // 128
                    O = [ps_o_p.tile([65, 512], f32, tag="o", name=f"O{h}")
                         for h in range(2)]
                    for kb in range(nkb):
                        qstart = max(0, 128 * kb - Q0)
                        # both heads' scores in one 2-bank psum tile; single
                        # exp over a [p, 2, width] AP halves ACT op overhead
                        ps_sc = ps_s_p.tile([128, 2 * QCH], f32, tag="s",
                                            name="ps_sc")
                        for h in range(2):
                            hp = slice(64 * h, 64 * h + 64)
                            nc.tensor.matmul(
                                ps_sc[:, QCH * h + qstart:QCH * h + QCH],
                                KT_res[hp, kb * 128:(kb + 1) * 128],
                                QT_cur[hp, qstart:QCH],
                                start=True, stop=True)
                        pt = ptp.tile([128, 2 * QCH], f32r, tag="pt",
                                      name="pt")
                        sc4 = ps_sc[:].rearrange("p (h q) -> p h q", h=2)
                        pt4 = pt[:].rearrange("p (h q) -> p h q", h=2)
                        nc.scalar.activation(pt4[:, :, qstart:QCH],
                                             sc4[:, :, qstart:QCH],
                                             AF.Exp, bias=0.0, scale=0.125)
                        if 128 * kb >= Q0:
                            ds = 128 * kb - Q0
                            for h in range(2):
                                nc.gpsimd.affine_select(
                                    out=pt[:, QCH * h + ds:QCH * h + ds + 128],
                                    in_=pt[:, QCH * h + ds:QCH * h + ds + 128],
                                    pattern=[[1, 128]], compare_op=AL.is_ge,
                                    fill=0.0, base=0, channel_multiplier=-1)
                        for h in range(2):
                            nc.tensor.matmul(
                                O[h][:, qstart:QCH],
                                V_res[:, 130 * kb + 65 * h:
                                      130 * kb + 65 * h + 65],
                                pt[:, QCH * h + qstart:QCH * h + QCH],
                                start=(kb == 0), stop=(kb == nkb - 1))
                    prev = (O, t0)
            proj_part(prev, norm_part(prev))
    nc.compile()
    return nc


# ---------------------------------------------------------------- host side

def _rope_tables(T):
    inv_freq = 1.0 / (ROPE_BASE ** (np.arange(0, HD, 2, dtype=np.float64) / HD))
    pos = np.arange(T, dtype=np.float64)
    ang = np.outer(pos, inv_freq)          # [T, 32]
    cos = np.cos(ang).astype(np.float32)   # [T, 32]
    sin = np.sin(ang).astype(np.float32)
    jm32 = np.arange(128) % 32
    # feat-major: row r (feature), col t (within-batch position)
    cos_t = np.ascontiguousarray(cos[:, jm32].T)   # [128, T]
    sin_t = np.ascontiguousarray(sin[:, jm32].T)
    return cos_t, sin_t


def _psgn():
    p = np.zeros((HD, HD), np.float32)
    for i in range(32):
        p[i + 32, i] = -1.0   # out dim i (<32) = -in dim i+32
        p[i, i + 32] = 1.0    # out dim i+32   = +in dim i
    pf = np.zeros((128, 128), np.float32)
    pf[0:64, 0:64] = p        # head A block
    pf[64:128, 64:128] = p    # head B block
    return np.ascontiguousarray(pf)


def make_core_inputs(x, qkv_w, qkv_b, proj_w, NB, T):
    NTOK = NB * T
    xf = np.ascontiguousarray(
        np.asarray(x).reshape(NTOK, HIDDEN).astype(np.float32))
    cos_t, sin_t = _rope_tables(T)
    psgn = _psgn()
    in_maps = []
    for c in range(N_CORES):
        col = HD * H2 * c
        wq = qkv_w[:, col:col + 128]
        wk = qkv_w[:, HIDDEN + col:HIDDEN + col + 128]
        wv = qkv_w[:, 2 * HIDDEN + col:2 * HIDDEN + col + 128]
        wc = np.ascontiguousarray(
            np.concatenate([wq, wk, wv], axis=1).astype(np.float32))
        bq = qkv_b[col:col + 128]
        bk = qkv_b[HIDDEN + col:HIDDEN + col + 128]
        bv = qkv_b[2 * HIDDEN + col:2 * HIDDEN + col + 128]
        browc = np.ascontiguousarray(
            np.concatenate([bq, bk, bv])[None, :].astype(np.float32))
        pwc = np.ascontiguousarray(
            proj_w[col:col + 128, :].astype(np.float32))
        in_maps.append({
            "x": xf, "w": wc, "brow": browc, "psgn": psgn, "pw": pwc,
            "cos_t": cos_t, "sin_t": sin_t,
        })
    return in_maps


_PROGRAM_CACHE = {}


def _get_program(NB, T):
    key = (NB, T)
    if key not in _PROGRAM_CACHE:
        _PROGRAM_CACHE[key] = build_program(NB, T)
    return _PROGRAM_CACHE[key]


def run(x, qkv_w, qkv_b, proj_w, proj_b, NB, T, trace=False):
    nc = _get_program(NB, T)
    in_maps = make_core_inputs(x, qkv_w, qkv_b, proj_w, NB, T)
    res = bass_utils.run_bass_kernel_spmd(
        nc, in_maps, core_ids=list(range(N_CORES)), trace=trace)
    acc = res.results[0]["yT"].astype(np.float32).copy()
    for c in range(1, N_CORES):
        acc += res.results[c]["yT"]
    out = (acc.T.reshape(NB, T, HIDDEN)
           + np.asarray(proj_b)[None, None, :].astype(np.float32))
    return out, res


def kernel(x, qkv_w, qkv_b, proj_w, proj_b):
    x = np.asarray(x)
    B, L, D = x.shape
    out, _ = run(x, np.asarray(qkv_w), np.asarray(qkv_b),
                 np.asarray(proj_w), np.asarray(proj_b), NB=B, T=L)
    return out.astype(np.float32)



# revision 3
# speedup vs baseline: 1.0999x; 1.0999x over previous
"""Trainium2 Bass kernel for nn_CausalSelfAttention (B=4, L=2048, D=1024, H=16).

Sharding: 2 heads per core (tensor parallel) x 8 cores; every core sees all
B*L tokens.  Each core computes q/k/v for its 2 heads, runs causal attention,
and emits a partial projection yT_c = proj_w[rows_c].T @ O_c^T of shape
[D, B*L] in bf16.  The host sums the 8 partials in fp32, transposes, and adds
proj_b.

Key layout choices (all bf16 on device, fp32 accumulation in PSUM):
  - x is pre-transposed on the host to xT [D, B*L] bf16, so no PE transposes
    are needed: qT/kT [feat, tok] come direct from w_tile.T @ xT_tile, and
    tok-major V comes from xT_tile.T @ wv_tile.
  - RoPE without matmuls: the head-dim feature order is permuted on the host
    (pairs (j, j+32) land in the same 32-partition quadrant), so rotate_half
    is a single DVE stream_shuffle; the sign is baked into the sin table.
    rope(q) = q*cos + shuffle(q)*sin_signed  (2 TT mults + 1 TT add).
  - Attention S^T/P^T/O^T per head with [V|1] ones-column rowsum trick;
    1/rowsum via exp(-ln()) on ACT; causal diag via gpsimd affine_select.
  - Per-chunk software pipeline: normalize+projection of chunk i-1 are
    emitted inside chunk i so the PE never waits on the ACT/DVE chain, and
    S(kb+1) is emitted before O(kb) so exp latency is hidden.
"""

import numpy as np
import ml_dtypes

import concourse.bass as bass  # noqa: F401
import concourse.tile as tile
from concourse import mybir, bacc
from concourse import bass_utils

f32 = mybir.dt.float32
bf16 = mybir.dt.bfloat16
AL = mybir.AluOpType
AF = mybir.ActivationFunctionType

BF16 = ml_dtypes.bfloat16


class _Bacc(bacc.Bacc):
    """Bacc that pins all activations to the one table set holding both
    ln and exp (plus copy/identity), so the per-iteration Ln<->Exp pair
    doesn't thrash ACT_TABLE_LOADs (~2.7us each)."""

    def insert_act_table_loads(self):
        import bass_rust as _bass_rust
        from concourse.hw_specs import get_activation_tables

        has_activation = any(
            isinstance(i, mybir.InstActivation)
            for bb in self.main_func.blocks
            for i in bb.instructions
        )
        if not has_activation:
            return
        tables = [
            (k, v if k == "natural_log_exp_and_others" else set())
            for k, v in get_activation_tables(self.m.arch).items()
        ]
        _bass_rust.insert_act_table_loads(self, tables)


HIDDEN = 1024
HEADS = 16
HD = 64
ROPE_BASE = 10000.0
N_CORES = 8
H2 = 2            # heads per core
QCH = 512         # token chunk = attention q granule
DT = HIDDEN // 128  # 8 d tiles

# within-head feature permutation: rope pairs (j, j+32) share a quadrant
PERM64 = list(range(0, 16)) + list(range(32, 48)) + \
    list(range(16, 32)) + list(range(48, 64))
# stream_shuffle mask: swap 16-halves within each 32-partition quadrant
SHUF_MASK = [(i + 16) % 32 for i in range(32)]


def build_program(NB, T, has_qkv_bias):
    """Build the per-core Bass program: NB batches of T tokens each."""
    assert T % QCH == 0
    NTOK = NB * T
    NKT = T // 128  # k tiles per batch
    nc = _Bacc("TRN2", target_bir_lowering=False, debug=False,
               num_devices=N_CORES)

    xT = nc.dram_tensor("xT", [HIDDEN, NTOK], bf16, kind="ExternalInput").ap()
    wqk = nc.dram_tensor("wqk", [128, DT * 256], bf16,
                         kind="ExternalInput").ap()
    wv = nc.dram_tensor("wv", [128, DT * 128], bf16,
                        kind="ExternalInput").ap()
    pw = nc.dram_tensor("pw", [128, HIDDEN], bf16, kind="ExternalInput").ap()
    cos_t = nc.dram_tensor("cos_t", [128, T], bf16, kind="ExternalInput").ap()
    sinm_t = nc.dram_tensor("sinm_t", [128, T], bf16,
                            kind="ExternalInput").ap()
    if has_qkv_bias:
        bqk = nc.dram_tensor("bqk", [1, 256], bf16, kind="ExternalInput").ap()
        bv = nc.dram_tensor("bv", [1, 128], bf16, kind="ExternalInput").ap()
    yT = nc.dram_tensor("yT", [HIDDEN, NTOK], bf16, kind="ExternalOutput").ap()

    with tile.TileContext(nc) as tc:
        with tc.tile_pool(name="const", bufs=1) as constp, \
             tc.tile_pool(name="resident", bufs=1) as resp, \
             tc.tile_pool(name="xload", bufs=24) as xp, \
             tc.tile_pool(name="rope", bufs=2) as ropep, \
             tc.tile_pool(name="qtcur", bufs=2) as qtp, \
             tc.tile_pool(name="pt", bufs=6) as ptp, \
             tc.tile_pool(name="ot", bufs=2) as otp, \
             tc.tile_pool(name="ysb", bufs=4) as yp, \
             tc.tile_pool(name="small", bufs=4) as smp, \
             tc.tile_pool(name="ps_s", bufs=3, space="PSUM") as ps_s_p, \
             tc.tile_pool(name="ps_o", bufs=2, space="PSUM") as ps_o_p, \
             tc.tile_pool(name="ps_m", bufs=3, space="PSUM") as ps_m_p:

            # ---- constants / residents ----
            wqk_sb = constp.tile([128, DT * 256], bf16)
            nc.sync.dma_start(wqk_sb[:], wqk[:])
            wv_sb = constp.tile([128, DT * 128], bf16)
            nc.sync.dma_start(wv_sb[:], wv[:])
            pw_sb = constp.tile([128, HIDDEN], bf16)
            nc.sync.dma_start(pw_sb[:], pw[:])
            cos_sb = constp.tile([128, T], bf16)
            nc.sync.dma_start(cos_sb[:], cos_t[:])
            sinm_sb = constp.tile([128, T], bf16)
            nc.sync.dma_start(sinm_sb[:], sinm_t[:])
            if has_qkv_bias:
                bqk_sb = constp.tile([1, 256], bf16)
                nc.sync.dma_start(bqk_sb[:], bqk[:])
                bv_sb = constp.tile([1, 128], bf16)
                nc.sync.dma_start(bv_sb[:], bv[:])
                ones_row = constp.tile([1, 512], bf16)
                nc.gpsimd.memset(ones_row[:], 1.0)
                ones_128 = constp.tile([1, 128], bf16)
                nc.gpsimd.memset(ones_128[:], 1.0)

            KT_res = resp.tile([128, T], bf16)
            V_res = resp.tile([128, NKT * 130], bf16)
            v4 = V_res[:].rearrange("p (kt h c) -> p kt h c", kt=NKT, h=2)
            nc.gpsimd.memset(v4[:, :, :, 64], 1.0)

            def norm_part(st):
                O, t0v = st
                ot_full = otp.tile([128, 512], bf16, tag="ot", name="ot_full")
                for h in range(2):
                    # 1/rowsum = exp(-ln(rowsum)) on ACT (same table set as
                    # the attention Exp)
                    lnv = smp.tile([1, 512], f32, tag="ln", name="lnv")
                    nc.scalar.activation(lnv[:], O[h][64:65, :], AF.Ln)
                    rs_sb = smp.tile([1, 512], f32, tag="rs", name="rs")
                    nc.scalar.activation(rs_sb[:], lnv[:], AF.Exp,
                                         bias=0.0, scale=-1.0)
                    rsb = smp.tile([64, 512], f32, tag="rsb", name="rsb")
                    nc.gpsimd.partition_broadcast(rsb[:], rs_sb[:])
                    nc.vector.tensor_tensor(ot_full[64 * h:64 * h + 64, :],
                                            O[h][0:64, :], rsb[:], AL.mult)
                return ot_full

            def proj_part(st, ot_full):
                O, t0v = st
                for oi in range(8):
                    ps_y = ps_m_p.tile([128, 512], f32, tag="m", name="ps_y")
                    nc.tensor.matmul(
                        ps_y[:], pw_sb[:, oi * 128:(oi + 1) * 128],
                        ot_full[:], start=True, stop=True)
                    ysb = yp.tile([128, 512], bf16, tag="y", name="ysb")
                    if oi % 2 == 0:
                        nc.vector.tensor_copy(ysb[:], ps_y[:])
                    else:
                        nc.scalar.copy(ysb[:], ps_y[:])
                    nc.sync.dma_start(
                        yT[oi * 128:(oi + 1) * 128, t0v:t0v + 512], ysb[:])

            prev = None
            for b in range(NB):
                for qc in range(T // QCH):
                    Q0 = qc * QCH
                    t0 = b * T + Q0
                    prev_ots = norm_part(prev) if prev is not None else None
                    # ---------- qkv phase for tokens [t0, t0+512) ----------
                    xa = [xp.tile([128, QCH], bf16, tag="xa", name=f"xa{dt}")
                          for dt in range(DT)]
                    for dt in range(DT):
                        nc.sync.dma_start(
                            xa[dt][:],
                            xT[dt * 128:(dt + 1) * 128, t0:t0 + QCH])
                    QT_cur = qtp.tile([128, QCH], bf16, tag="qt", name="QT")
                    for f in range(2):  # 0=q, 1=k
                        ps_f = ps_m_p.tile([128, QCH], f32, tag="m",
                                           name="ps_f")
                        for dt in range(DT):
                            nc.tensor.matmul(
                                ps_f[:],
                                wqk_sb[:, dt * 256 + f * 128:
                                       dt * 256 + (f + 1) * 128],
                                xa[dt][:], start=(dt == 0),
                                stop=(dt == DT - 1 and not has_qkv_bias))
                        if has_qkv_bias:
                            nc.tensor.matmul(
                                ps_f[:], bqk_sb[:, f * 128:(f + 1) * 128],
                                ones_row[:], start=False, stop=True)
                        # RoPE: shuffle swaps the 16-halves of each quadrant
                        # (= rotate_half under the host-side PERM64 order)
                        shuf = ropep.tile([128, QCH], f32, tag="sh",
                                          name="shuf")
                        nc.vector.stream_shuffle(shuf[:], ps_f[:], SHUF_MASK)
                        t1 = ropep.tile([128, QCH], bf16, tag="t1", name="t1")
                        nc.vector.tensor_tensor(
                            t1[:], ps_f[:], cos_sb[:, Q0:Q0 + QCH], AL.mult)
                        t2 = ropep.tile([128, QCH], bf16, tag="t2", name="t2")
                        nc.gpsimd.tensor_tensor(
                            t2[:], shuf[:], sinm_sb[:, Q0:Q0 + QCH], AL.mult)
                        dst = (QT_cur[:] if f == 0
                               else KT_res[:, Q0:Q0 + QCH])
                        nc.vector.tensor_tensor(dst, t1[:], t2[:], AL.add)
                    # V in token-major layout, one psum bank, 4 tok blocks
                    ps_v = ps_m_p.tile([128, QCH], f32, tag="m", name="ps_v")
                    for tt in range(4):
                        sl = slice(tt * 128, (tt + 1) * 128)
                        for dt in range(DT):
                            nc.tensor.matmul(
                                ps_v[:, sl], xa[dt][:, sl],
                                wv_sb[:, dt * 128:(dt + 1) * 128],
                                start=(dt == 0),
                                stop=(dt == DT - 1 and not has_qkv_bias))
                        if has_qkv_bias:
                            nc.tensor.matmul(ps_v[:, sl], ones_128[:],
                                             bv_sb[:], start=False, stop=True)
                    kt0 = Q0 // 128
                    nc.vector.tensor_copy(
                        v4[:, kt0:kt0 + 4, :, 0:64],
                        ps_v[:].rearrange("p (tt h j) -> p tt h j",
                                          tt=4, h=2))
                    # ---------- deferred projection of chunk i-1 ----------
                    if prev is not None:
                        proj_part(prev, prev_ots)
                    # ---------- attention for (b, qc) ----------
                    nkb = kt0 + 4
                    O = [ps_o_p.tile([65, 512], f32, tag="o", name=f"O{h}")
                         for h in range(2)]
                    # diag blocks first (gives affine_select extra slack),
                    # then past blocks; S one kb ahead of O hides exp latency
                    kb_order = list(range(kt0, nkb)) + list(range(0, kt0))
                    pts = {}

                    def emit_S(kb):
                        qstart = max(0, 128 * kb - Q0)
                        pth = []
                        for h in range(2):
                            hp = slice(64 * h, 64 * h + 64)
                            ps_sc = ps_s_p.tile([128, 512], f32, tag="s",
                                                name="ps_sc")
                            nc.tensor.matmul(
                                ps_sc[:, qstart:QCH],
                                KT_res[hp, kb * 128:(kb + 1) * 128],
                                QT_cur[hp, qstart:QCH],
                                start=True, stop=True)
                            pt = ptp.tile([128, QCH], bf16, tag="pt",
                                          name="pt")
                            nc.scalar.activation(pt[:, qstart:QCH],
                                                 ps_sc[:, qstart:QCH],
                                                 AF.Exp, bias=0.0,
                                                 scale=0.125)
                            if 128 * kb >= Q0:
                                ds = 128 * kb - Q0
                                nc.gpsimd.affine_select(
                                    out=pt[:, ds:ds + 128],
                                    in_=pt[:, ds:ds + 128],
                                    pattern=[[1, 128]], compare_op=AL.is_ge,
                                    fill=0.0, base=0, channel_multiplier=-1)
                            pth.append(pt)
                        pts[kb] = pth

                    def emit_O(kb, first, last):
                        qstart = max(0, 128 * kb - Q0)
                        pth = pts.pop(kb)
                        for h in range(2):
                            nc.tensor.matmul(
                                O[h][:, qstart:QCH],
                                V_res[:, 130 * kb + 65 * h:
                                      130 * kb + 65 * h + 65],
                                pth[h][:, qstart:QCH],
                                start=first, stop=last)

                    for i, kb in enumerate(kb_order):
                        emit_S(kb)
                        if i >= 1:
                            emit_O(kb_order[i - 1], first=(i == 1),
                                   last=False)
                    emit_O(kb_order[-1], first=(len(kb_order) == 1),
                           last=True)
                    prev = (O, t0)
            proj_part(prev, norm_part(prev))
    nc.compile()
    return nc


# ---------------------------------------------------------------- host side

def _rope_tables(T):
    """cos / sign-baked-sin tables in the PERM64 feature order, bf16."""
    inv_freq = 1.0 / (ROPE_BASE ** (np.arange(0, HD, 2, dtype=np.float64)
                                    / HD))
    pos = np.arange(T, dtype=np.float64)
    ang = np.outer(pos, inv_freq)          # [T, 32]
    cos = np.cos(ang)                      # [T, 32]
    sin = np.sin(ang)
    p = np.arange(128)
    d = np.asarray(PERM64)[p % 64]         # original dim held by partition p
    cos_t = cos[:, d % 32].T               # [128, T]
    sgn = np.where(d < 32, -1.0, 1.0)
    sinm_t = (sin[:, d % 32] * sgn[None, :]).T
    return (np.ascontiguousarray(cos_t).astype(BF16),
            np.ascontiguousarray(sinm_t).astype(BF16))


def make_core_inputs(x, qkv_w, qkv_b, proj_w, NB, T, has_qkv_bias):
    NTOK = NB * T
    xT = np.asarray(x).reshape(NTOK, HIDDEN).T.astype(BF16, order='C')
    cos_t, sinm_t = _rope_tables(T)
    qkv_w = np.asarray(qkv_w)
    qkv_b = np.asarray(qkv_b)
    perm = np.asarray(PERM64)
    in_maps = []
    for c in range(N_CORES):
        col = HD * H2 * c
        # q/k columns in PERM64 order (per head), v natural
        qcols = np.concatenate([col + perm, col + 64 + perm])
        wq = qkv_w[:, qcols]
        wk = qkv_w[:, HIDDEN + qcols]
        wv = qkv_w[:, 2 * HIDDEN + col:2 * HIDDEN + col + 128]
        wqk_t = np.zeros((128, DT * 256), dtype=np.float32)
        wv_t = np.zeros((128, DT * 128), dtype=np.float32)
        for dt in range(DT):
            rows = slice(dt * 128, (dt + 1) * 128)
            wqk_t[:, dt * 256:dt * 256 + 128] = wq[rows]
            wqk_t[:, dt * 256 + 128:dt * 256 + 256] = wk[rows]
            wv_t[:, dt * 128:(dt + 1) * 128] = wv[rows]
        pwc = np.ascontiguousarray(
            proj_w[col:col + 128, :]).astype(BF16)
        m = {
            "xT": xT, "wqk": wqk_t.astype(BF16), "wv": wv_t.astype(BF16),
            "pw": pwc, "cos_t": cos_t, "sinm_t": sinm_t,
        }
        if has_qkv_bias:
            m["bqk"] = np.concatenate(
                [qkv_b[qcols], qkv_b[HIDDEN + qcols]])[None, :].astype(BF16)
            m["bv"] = qkv_b[2 * HIDDEN + col:
                            2 * HIDDEN + col + 128][None, :].astype(BF16)
        in_maps.append(m)
    return in_maps


_PROGRAM_CACHE = {}


def _get_program(NB, T, has_qkv_bias):
    key = (NB, T, has_qkv_bias)
    if key not in _PROGRAM_CACHE:
        _PROGRAM_CACHE[key] = build_program(NB, T, has_qkv_bias)
    return _PROGRAM_CACHE[key]


def run(x, qkv_w, qkv_b, proj_w, proj_b, NB, T, trace=False):
    has_qkv_bias = bool(np.any(np.asarray(qkv_b)))
    nc = _get_program(NB, T, has_qkv_bias)
    in_maps = make_core_inputs(x, qkv_w, qkv_b, proj_w, NB, T, has_qkv_bias)
    res = bass_utils.run_bass_kernel_spmd(
        nc, in_maps, core_ids=list(range(N_CORES)), trace=trace)
    acc = res.results[0]["yT"].astype(np.float32)
    for c in range(1, N_CORES):
        acc += res.results[c]["yT"].astype(np.float32)
    out = (acc.T.reshape(NB, T, HIDDEN)
           + np.asarray(proj_b)[None, None, :].astype(np.float32))
    return out, res


def kernel(x, qkv_w, qkv_b, proj_w, proj_b):
    x = np.asarray(x)
    B, L, D = x.shape
    out, _ = run(x, np.asarray(qkv_w), np.asarray(qkv_b),
                 np.asarray(proj_w), np.asarray(proj_b), NB=B, T=L)
    return out.astype(np.float32)


# revision 8
# speedup vs baseline: 1.1616x; 1.0561x over previous
"""Trainium2 Bass kernel for nn_CausalSelfAttention (B=4, L=2048, D=1024, H=16).

Sharding: 2 heads per core (tensor parallel) x 8 cores; every core sees all
B*L tokens.  Each core computes q/k/v for its 2 heads, runs causal attention,
and emits a partial projection y_c = proj_w[rows_c].T @ O_c of shape
[B*L, D] (chunk-tiled) in bf16.  The host sums the 8 partials in fp32 and
adds proj_b.

Key layout choices (all bf16 on device, fp32 accumulation in PSUM):
  - x is pre-transposed AND chunk-tiled on the host (xTt[p, ch, dt, c] =
    x[ch*512+c, dt*128+p]), so each 512-token chunk is ONE contiguous-line
    DMA (8KB/partition) and no PE transposes are needed.  Output is tiled
    the same way: one 8KB-line store per chunk.
  - RoPE without matmuls: the head-dim feature order is permuted on the host
    (pairs (j, j+32) land in the same 32-partition quadrant), so rotate_half
    is a single DVE stream_shuffle; the sign is baked into the sin table.
    rope(q) = q*cos + shuffle(q)*sin_signed.
  - Attention S^T/P^T/O^T per head with [V|1] ones-column rowsum trick;
    1/rowsum via DVE reciprocal_approx_fast + stride-0 broadcast AP;
    causal diag via gpsimd affine_select.
  - Chunks are BATCH-INTERLEAVED ((b0,qc0),(b1,qc0),...,(b0,qc1),...) with
    per-batch K/V residents, so the small-attention batch-start chunks don't
    stall the pipe; normalize+projection of chunk i-1 are emitted inside
    chunk i, and S(kb+1) is emitted before O(kb) to hide exp latency.
"""

import numpy as np
import ml_dtypes

import concourse.bass as bass  # noqa: F401
import concourse.tile as tile
from concourse import mybir, bacc
from concourse import bass_utils

f32 = mybir.dt.float32
bf16 = mybir.dt.bfloat16
AL = mybir.AluOpType
AF = mybir.ActivationFunctionType

BF16 = ml_dtypes.bfloat16


class _Bacc(bacc.Bacc):
    """Bacc that pins all activations to the one table set holding exp, so
    ACT_TABLE_LOADs (~2.7us each) happen once."""

    def insert_act_table_loads(self):
        import bass_rust as _bass_rust
        from concourse.hw_specs import get_activation_tables

        has_activation = any(
            isinstance(i, mybir.InstActivation)
            for bb in self.main_func.blocks
            for i in bb.instructions
        )
        if not has_activation:
            return
        tables = [
            (k, v if k == "natural_log_exp_and_others" else set())
            for k, v in get_activation_tables(self.m.arch).items()
        ]
        _bass_rust.insert_act_table_loads(self, tables)


HIDDEN = 1024
HEADS = 16
HD = 64
ROPE_BASE = 10000.0
N_CORES = 8
H2 = 2            # heads per core
QCH = 512         # token chunk = attention q granule
DT = HIDDEN // 128  # 8 d tiles
CHW = DT * QCH    # flattened chunk width in the tiled x / y layouts

# within-head feature permutation: rope pairs (j, j+32) share a quadrant
PERM64 = list(range(0, 16)) + list(range(32, 48)) + \
    list(range(16, 32)) + list(range(48, 64))
# stream_shuffle mask: swap 16-halves within each 32-partition quadrant
SHUF_MASK = [(i + 16) % 32 for i in range(32)]


def build_program(NB, T, has_qkv_bias):
    """Build the per-core Bass program: NB batches of T tokens each."""
    assert T % QCH == 0
    NCHB = T // QCH          # chunks per batch
    NCH = NB * NCHB          # total chunks
    NKT = T // 128           # k tiles per batch
    nc = _Bacc("TRN2", target_bir_lowering=False, debug=False,
               num_devices=N_CORES)

    xTt = nc.dram_tensor("xTt", [128, NCH * CHW], bf16,
                         kind="ExternalInput").ap()
    wqk = nc.dram_tensor("wqk", [128, DT * 256], bf16,
                         kind="ExternalInput").ap()
    wv = nc.dram_tensor("wv", [128, DT * 128], bf16,
                        kind="ExternalInput").ap()
    pw = nc.dram_tensor("pw", [128, HIDDEN], bf16, kind="ExternalInput").ap()
    cos_t = nc.dram_tensor("cos_t", [128, T], bf16, kind="ExternalInput").ap()
    sinm_t = nc.dram_tensor("sinm_t", [128, T], bf16,
                            kind="ExternalInput").ap()
    if has_qkv_bias:
        bqk = nc.dram_tensor("bqk", [1, 256], bf16, kind="ExternalInput").ap()
        bv = nc.dram_tensor("bv", [1, 128], bf16, kind="ExternalInput").ap()
    yTt = nc.dram_tensor("yTt", [128, NCH * CHW], bf16,
                         kind="ExternalOutput").ap()

    # batch-interleaved chunk order; ch = b * NCHB + qc indexes the tiles
    chunk_seq = [(b, qc) for qc in range(NCHB) for b in range(NB)]

    with tile.TileContext(nc) as tc:
        with tc.tile_pool(name="const", bufs=1) as constp, \
             tc.tile_pool(name="resident", bufs=1) as resp, \
             tc.tile_pool(name="xload", bufs=3) as xp, \
             tc.tile_pool(name="rope", bufs=2) as ropep, \
             tc.tile_pool(name="qtcur", bufs=2) as qtp, \
             tc.tile_pool(name="pt", bufs=6) as ptp, \
             tc.tile_pool(name="ot", bufs=2) as otp, \
             tc.tile_pool(name="ysb", bufs=2) as yp, \
             tc.tile_pool(name="small", bufs=4) as smp, \
             tc.tile_pool(name="ps_s", bufs=3, space="PSUM") as ps_s_p, \
             tc.tile_pool(name="ps_o", bufs=2, space="PSUM") as ps_o_p, \
             tc.tile_pool(name="ps_m", bufs=3, space="PSUM") as ps_m_p:

            # ---- constants / residents ----
            wqk_sb = constp.tile([128, DT * 256], bf16)
            nc.sync.dma_start(wqk_sb[:], wqk[:])
            wv_sb = constp.tile([128, DT * 128], bf16)
            nc.sync.dma_start(wv_sb[:], wv[:])
            pw_sb = constp.tile([128, HIDDEN], bf16)
            nc.sync.dma_start(pw_sb[:], pw[:])
            cos_sb = constp.tile([128, T], bf16)
            nc.sync.dma_start(cos_sb[:], cos_t[:])
            sinm_sb = constp.tile([128, T], bf16)
            nc.sync.dma_start(sinm_sb[:], sinm_t[:])
            if has_qkv_bias:
                bqk_sb = constp.tile([1, 256], bf16)
                nc.sync.dma_start(bqk_sb[:], bqk[:])
                bv_sb = constp.tile([1, 128], bf16)
                nc.sync.dma_start(bv_sb[:], bv[:])
                ones_row = constp.tile([1, 512], bf16)
                nc.gpsimd.memset(ones_row[:], 1.0)
                ones_128 = constp.tile([1, 128], bf16)
                nc.gpsimd.memset(ones_128[:], 1.0)

            KT_res = [resp.tile([128, T], bf16, name=f"KT{b}")
                      for b in range(NB)]
            v4 = []
            for b in range(NB):
                V_res = resp.tile([128, NKT * 130], bf16, name=f"V{b}")
                v4b = V_res[:].rearrange("p (kt h c) -> p kt h c",
                                         kt=NKT, h=2)
                nc.gpsimd.memset(v4b[:, :, :, 64], 1.0)
                v4.append(v4b)

            def load_chunk(ch):
                xa = xp.tile([128, CHW], bf16, tag="xa", name="xa")
                nc.sync.dma_start(xa[:], xTt[:, ch * CHW:(ch + 1) * CHW])
                return xa

            def norm_part(st):
                O, ch = st
                ot_full = otp.tile([128, 512], bf16, tag="ot", name="ot_full")
                for h in range(2):
                    # 1/rowsum = exp(-ln(rowsum)) on ACT (same table set as
                    # the attention Exp)
                    lnv = smp.tile([1, 512], f32, tag="ln", name="lnv")
                    nc.scalar.activation(lnv[:], O[h][64:65, :], AF.Ln)
                    rs_sb = smp.tile([1, 512], f32, tag="rs", name="rs")
                    nc.scalar.activation(rs_sb[:], lnv[:], AF.Exp,
                                         bias=0.0, scale=-1.0)
                    rsb = smp.tile([64, 512], f32, tag="rsb", name="rsb")
                    nc.gpsimd.partition_broadcast(rsb[:], rs_sb[:])
                    nc.vector.tensor_tensor(
                        ot_full[64 * h:64 * h + 64, :],
                        O[h][0:64, :], rsb[:], AL.mult)
                return ot_full

            def proj_part(st, ot_full):
                O, ch = st
                ysb = yp.tile([128, CHW], bf16, tag="y", name="ysb")
                for oi in range(8):
                    ps_y = ps_m_p.tile([128, 512], f32, tag="m", name="ps_y")
                    nc.tensor.matmul(
                        ps_y[:], pw_sb[:, oi * 128:(oi + 1) * 128],
                        ot_full[:], start=True, stop=True)
                    nc.vector.tensor_copy(
                        ysb[:, oi * 512:(oi + 1) * 512], ps_y[:])
                nc.sync.dma_start(yTt[:, ch * CHW:(ch + 1) * CHW], ysb[:])

            prev = None
            xa_cur = load_chunk(chunk_seq[0][0] * NCHB + chunk_seq[0][1])
            for ci, (b, qc) in enumerate(chunk_seq):
                Q0 = qc * QCH
                ch = b * NCHB + qc
                xa = xa_cur
                if ci + 1 < len(chunk_seq):
                    nb_, nqc_ = chunk_seq[ci + 1]
                    xa_cur = load_chunk(nb_ * NCHB + nqc_)
                prev_ots = norm_part(prev) if prev is not None else None
                # ---------- qkv phase ----------
                QT_cur = qtp.tile([128, QCH], bf16, tag="qt", name="QT")
                for f in range(2):  # 0=q, 1=k
                    ps_f = ps_m_p.tile([128, QCH], f32, tag="m", name="ps_f")
                    for dt in range(DT):
                        nc.tensor.matmul(
                            ps_f[:],
                            wqk_sb[:, dt * 256 + f * 128:
                                   dt * 256 + (f + 1) * 128],
                            xa[:, dt * 512:(dt + 1) * 512],
                            start=(dt == 0),
                            stop=(dt == DT - 1 and not has_qkv_bias))
                    if has_qkv_bias:
                        nc.tensor.matmul(
                            ps_f[:], bqk_sb[:, f * 128:(f + 1) * 128],
                            ones_row[:], start=False, stop=True)
                    # RoPE: shuffle swaps the 16-halves of each quadrant
                    # (= rotate_half under the host-side PERM64 order)
                    shuf = ropep.tile([128, QCH], f32, tag="sh", name="shuf")
                    nc.vector.stream_shuffle(shuf[:], ps_f[:], SHUF_MASK)
                    t1 = ropep.tile([128, QCH], bf16, tag="t1", name="t1")
                    nc.vector.tensor_tensor(
                        t1[:], ps_f[:], cos_sb[:, Q0:Q0 + QCH], AL.mult)
                    t2 = ropep.tile([128, QCH], bf16, tag="t2", name="t2")
                    nc.vector.tensor_tensor(
                        t2[:], shuf[:], sinm_sb[:, Q0:Q0 + QCH], AL.mult)
                    dst = (QT_cur[:] if f == 0
                           else KT_res[b][:, Q0:Q0 + QCH])
                    nc.vector.tensor_tensor(dst, t1[:], t2[:], AL.add)
                # V in token-major layout, one psum bank, 4 tok blocks
                ps_v = ps_m_p.tile([128, QCH], f32, tag="m", name="ps_v")
                for tt in range(4):
                    sl = slice(tt * 128, (tt + 1) * 128)
                    for dt in range(DT):
                        nc.tensor.matmul(
                            ps_v[:, sl],
                            xa[:, dt * 512 + tt * 128:
                               dt * 512 + (tt + 1) * 128],
                            wv_sb[:, dt * 128:(dt + 1) * 128],
                            start=(dt == 0),
                            stop=(dt == DT - 1 and not has_qkv_bias))
                    if has_qkv_bias:
                        nc.tensor.matmul(ps_v[:, sl], ones_128[:],
                                         bv_sb[:], start=False, stop=True)
                kt0 = Q0 // 128
                nc.vector.tensor_copy(
                    v4[b][:, kt0:kt0 + 4, :, 0:64],
                    ps_v[:].rearrange("p (tt h j) -> p tt h j", tt=4, h=2))
                # ---------- deferred projection of chunk i-1 ----------
                if prev is not None:
                    proj_part(prev, prev_ots)
                # ---------- attention for (b, qc) ----------
                nkb = kt0 + 4
                O = [ps_o_p.tile([65, 512], f32, tag="o", name=f"O{h}")
                     for h in range(2)]
                # diag blocks first (gives affine_select extra slack),
                # then past blocks; S one kb ahead of O hides exp latency
                kb_order = list(range(kt0, nkb)) + list(range(0, kt0))
                pts = {}

                def emit_S(kb):
                    qstart = max(0, 128 * kb - Q0)
                    pth = []
                    for h in range(2):
                        hp = slice(64 * h, 64 * h + 64)
                        ps_sc = ps_s_p.tile([128, 512], f32, tag="s",
                                            name="ps_sc")
                        nc.tensor.matmul(
                            ps_sc[:, qstart:QCH],
                            KT_res[b][hp, kb * 128:(kb + 1) * 128],
                            QT_cur[hp, qstart:QCH],
                            start=True, stop=True)
                        pt = ptp.tile([128, QCH], bf16, tag="pt", name="pt")
                        nc.scalar.activation(pt[:, qstart:QCH],
                                             ps_sc[:, qstart:QCH],
                                             AF.Exp, bias=0.0, scale=0.125)
                        if 128 * kb >= Q0:
                            ds = 128 * kb - Q0
                            nc.gpsimd.affine_select(
                                out=pt[:, ds:ds + 128],
                                in_=pt[:, ds:ds + 128],
                                pattern=[[1, 128]], compare_op=AL.is_ge,
                                fill=0.0, base=0, channel_multiplier=-1)
                        pth.append(pt)
                    pts[kb] = pth

                def emit_O(kb, first, last):
                    qstart = max(0, 128 * kb - Q0)
                    pth = pts.pop(kb)
                    for h in range(2):
                        nc.tensor.matmul(
                            O[h][:, qstart:QCH],
                            V_res_ap(b, kb, h),
                            pth[h][:, qstart:QCH],
                            start=first, stop=last)

                def V_res_ap(b_, kb, h):
                    return v4[b_][:, kb, h, :]

                for i, kb in enumerate(kb_order):
                    emit_S(kb)
                    if i >= 1:
                        emit_O(kb_order[i - 1], first=(i == 1), last=False)
                emit_O(kb_order[-1], first=(len(kb_order) == 1), last=True)
                prev = (O, ch)
            proj_part(prev, norm_part(prev))
    nc.compile()
    return nc


# ---------------------------------------------------------------- host side

def _rope_tables(T):
    """cos / sign-baked-sin tables in the PERM64 feature order, bf16."""
    inv_freq = 1.0 / (ROPE_BASE ** (np.arange(0, HD, 2, dtype=np.float64)
                                    / HD))
    pos = np.arange(T, dtype=np.float64)
    ang = np.outer(pos, inv_freq)          # [T, 32]
    cos = np.cos(ang)                      # [T, 32]
    sin = np.sin(ang)
    p = np.arange(128)
    d = np.asarray(PERM64)[p % 64]         # original dim held by partition p
    cos_t = cos[:, d % 32].T               # [128, T]
    sgn = np.where(d < 32, -1.0, 1.0)
    sinm_t = (sin[:, d % 32] * sgn[None, :]).T
    return (np.ascontiguousarray(cos_t).astype(BF16),
            np.ascontiguousarray(sinm_t).astype(BF16))


def make_core_inputs(x, qkv_w, qkv_b, proj_w, NB, T, has_qkv_bias):
    NTOK = NB * T
    NCH = NTOK // QCH
    xf = np.asarray(x).reshape(NTOK, HIDDEN).astype(BF16)
    # xTt[p, ch, dt, c] = x[ch*512+c, dt*128+p]
    xTt = np.ascontiguousarray(
        xf.reshape(NCH, QCH, DT, 128).transpose(3, 0, 2, 1)
    ).reshape(128, NCH * CHW)
    cos_t, sinm_t = _rope_tables(T)
    qkv_w = np.asarray(qkv_w)
    qkv_b = np.asarray(qkv_b)
    perm = np.asarray(PERM64)
    in_maps = []
    for c in range(N_CORES):
        col = HD * H2 * c
        # q/k columns in PERM64 order (per head), v natural
        qcols = np.concatenate([col + perm, col + 64 + perm])
        wq = qkv_w[:, qcols]
        wk = qkv_w[:, HIDDEN + qcols]
        wv = qkv_w[:, 2 * HIDDEN + col:2 * HIDDEN + col + 128]
        wqk_t = np.zeros((128, DT * 256), dtype=np.float32)
        wv_t = np.zeros((128, DT * 128), dtype=np.float32)
        for dt in range(DT):
            rows = slice(dt * 128, (dt + 1) * 128)
            wqk_t[:, dt * 256:dt * 256 + 128] = wq[rows]
            wqk_t[:, dt * 256 + 128:dt * 256 + 256] = wk[rows]
            wv_t[:, dt * 128:(dt + 1) * 128] = wv[rows]
        pwc = np.ascontiguousarray(
            proj_w[col:col + 128, :]).astype(BF16)
        m = {
            "xTt": xTt, "wqk": wqk_t.astype(BF16), "wv": wv_t.astype(BF16),
            "pw": pwc, "cos_t": cos_t, "sinm_t": sinm_t,
        }
        if has_qkv_bias:
            m["bqk"] = np.concatenate(
                [qkv_b[qcols], qkv_b[HIDDEN + qcols]])[None, :].astype(BF16)
            m["bv"] = qkv_b[2 * HIDDEN + col:
                            2 * HIDDEN + col + 128][None, :].astype(BF16)
        in_maps.append(m)
    return in_maps


_PROGRAM_CACHE = {}


def _get_program(NB, T, has_qkv_bias):
    key = (NB, T, has_qkv_bias)
    if key not in _PROGRAM_CACHE:
        _PROGRAM_CACHE[key] = build_program(NB, T, has_qkv_bias)
    return _PROGRAM_CACHE[key]


def run(x, qkv_w, qkv_b, proj_w, proj_b, NB, T, trace=False):
    has_qkv_bias = bool(np.any(np.asarray(qkv_b)))
    nc = _get_program(NB, T, has_qkv_bias)
    in_maps = make_core_inputs(x, qkv_w, qkv_b, proj_w, NB, T, has_qkv_bias)
    res = bass_utils.run_bass_kernel_spmd(
        nc, in_maps, core_ids=list(range(N_CORES)), trace=trace)
    acc = res.results[0]["yTt"].astype(np.float32)
    for c in range(1, N_CORES):
        acc += res.results[c]["yTt"].astype(np.float32)
    NTOK = NB * T
    NCH = NTOK // QCH
    # y[ch*512+c, oi*128+p] = acc[p, ch, oi, c]
    y = acc.reshape(128, NCH, DT, QCH).transpose(1, 3, 2, 0).reshape(
        NTOK, HIDDEN)
    out = (y.reshape(NB, T, HIDDEN)
           + np.asarray(proj_b)[None, None, :].astype(np.float32))
    return out, res


def kernel(x, qkv_w, qkv_b, proj_w, proj_b):
    x = np.asarray(x)
    B, L, D = x.shape
    out, _ = run(x, np.asarray(qkv_w), np.asarray(qkv_b),
                 np.asarray(proj_w), np.asarray(proj_b), NB=B, T=L)
    return out.astype(np.float32)


# revision 10
# speedup vs baseline: 1.5877x; 1.3669x over previous
"""Trainium2 Bass kernel for nn_CausalSelfAttention (B=4, L=2048, D=1024, H=16).

Sharding: 2 heads per core (tensor parallel) x 8 cores; every core sees all
B*L tokens.  Each core computes q/k/v for its 2 heads, runs causal attention,
and emits a partial projection y_c = proj_w[rows_c].T @ O_c of shape
[B*L, D] (chunk-tiled) in bf16.  The host sums the 8 partials in fp32 and
adds proj_b.

Key layout choices (all bf16 on device, fp32 accumulation in PSUM):
  - x is pre-transposed AND chunk-tiled on the host (xTt[p, ch, dt, c] =
    x[ch*512+c, dt*128+p]), so each 512-token chunk is ONE contiguous-line
    DMA (8KB/partition) and no PE transposes are needed.  Output is tiled
    the same way: one 8KB-line store per chunk.
  - RoPE without matmuls: the head-dim feature order is permuted on the host
    (pairs (j, j+32) land in the same 32-partition quadrant), so rotate_half
    is a single DVE stream_shuffle; the sign is baked into the sin table.
    rope(q) = q*cos + shuffle(q)*sin_signed.
  - Attention S^T/P^T/O^T per head with [V|1] ones-column rowsum trick;
    1/rowsum via DVE reciprocal_approx_fast + stride-0 broadcast AP;
    causal diag via gpsimd affine_select.
  - Chunks are BATCH-INTERLEAVED ((b0,qc0),(b1,qc0),...,(b0,qc1),...) with
    per-batch K/V residents, so the small-attention batch-start chunks don't
    stall the pipe; normalize+projection of chunk i-1 are emitted inside
    chunk i, and S(kb+1) is emitted before O(kb) to hide exp latency.
"""

import numpy as np
import ml_dtypes

import concourse.bass as bass  # noqa: F401
import concourse.tile as tile
from concourse import mybir, bacc
from concourse import bass_utils

f32 = mybir.dt.float32
bf16 = mybir.dt.bfloat16
AL = mybir.AluOpType
AF = mybir.ActivationFunctionType

BF16 = ml_dtypes.bfloat16


class _Bacc(bacc.Bacc):
    """Bacc that pins all activations to the one table set holding exp, so
    ACT_TABLE_LOADs (~2.7us each) happen once."""

    def insert_act_table_loads(self):
        import bass_rust as _bass_rust
        from concourse.hw_specs import get_activation_tables

        has_activation = any(
            isinstance(i, mybir.InstActivation)
            for bb in self.main_func.blocks
            for i in bb.instructions
        )
        if not has_activation:
            return
        tables = [
            (k, v if k == "natural_log_exp_and_others" else set())
            for k, v in get_activation_tables(self.m.arch).items()
        ]
        _bass_rust.insert_act_table_loads(self, tables)


HIDDEN = 1024
HEADS = 16
HD = 64
ROPE_BASE = 10000.0
N_CORES = 8
H2 = 2            # heads per core
QCH = 512         # token chunk = attention q granule
DT = HIDDEN // 128  # 8 d tiles
CHW = DT * QCH    # flattened chunk width in the tiled x / y layouts

# within-head feature permutation: rope pairs (j, j+32) share a quadrant
PERM64 = list(range(0, 16)) + list(range(32, 48)) + \
    list(range(16, 32)) + list(range(48, 64))
# stream_shuffle mask: swap 16-halves within each 32-partition quadrant
SHUF_MASK = [(i + 16) % 32 for i in range(32)]


def build_program(NB, T, has_qkv_bias):
    """Build the per-core Bass program: NB batches of T tokens each."""
    assert T % QCH == 0
    NCHB = T // QCH          # chunks per batch
    NCH = NB * NCHB          # total chunks
    NKT = T // 128           # k tiles per batch
    nc = _Bacc("TRN2", target_bir_lowering=False, debug=False,
               num_devices=N_CORES)

    xTt = nc.dram_tensor("xTt", [128, NCH * CHW], bf16,
                         kind="ExternalInput").ap()
    wqk = nc.dram_tensor("wqk", [128, DT * 256], bf16,
                         kind="ExternalInput").ap()
    wv = nc.dram_tensor("wv", [128, DT * 128], bf16,
                        kind="ExternalInput").ap()
    pw = nc.dram_tensor("pw", [128, HIDDEN], bf16, kind="ExternalInput").ap()
    cos_t = nc.dram_tensor("cos_t", [128, T], bf16, kind="ExternalInput").ap()
    sinm_t = nc.dram_tensor("sinm_t", [128, T], bf16,
                            kind="ExternalInput").ap()
    if has_qkv_bias:
        bqk = nc.dram_tensor("bqk", [1, 256], bf16, kind="ExternalInput").ap()
        bv = nc.dram_tensor("bv", [1, 128], bf16, kind="ExternalInput").ap()
    yTt = nc.dram_tensor("yTt", [128, NCH * CHW], bf16,
                         kind="ExternalOutput").ap()

    # batch-interleaved chunk order; ch = b * NCHB + qc indexes the tiles
    chunk_seq = [(b, qc) for qc in range(NCHB) for b in range(NB)]

    with tile.TileContext(nc) as tc:
        with tc.tile_pool(name="const", bufs=1) as constp, \
             tc.tile_pool(name="resident", bufs=1) as resp, \
             tc.tile_pool(name="xload", bufs=3) as xp, \
             tc.tile_pool(name="rope", bufs=2) as ropep, \
             tc.tile_pool(name="qtcur", bufs=2) as qtp, \
             tc.tile_pool(name="pt", bufs=6) as ptp, \
             tc.tile_pool(name="ot", bufs=2) as otp, \
             tc.tile_pool(name="ysb", bufs=2) as yp, \
             tc.tile_pool(name="small", bufs=4) as smp, \
             tc.tile_pool(name="ps_s", bufs=3, space="PSUM") as ps_s_p, \
             tc.tile_pool(name="ps_o", bufs=2, space="PSUM") as ps_o_p, \
             tc.tile_pool(name="ps_m", bufs=3, space="PSUM") as ps_m_p:

            # ---- constants / residents ----
            wqk_sb = constp.tile([128, DT * 256], bf16)
            nc.sync.dma_start(wqk_sb[:], wqk[:])
            wv_sb = constp.tile([128, DT * 128], bf16)
            nc.sync.dma_start(wv_sb[:], wv[:])
            pw_sb = constp.tile([128, HIDDEN], bf16)
            nc.sync.dma_start(pw_sb[:], pw[:])
            cos_sb = constp.tile([128, T], bf16)
            nc.sync.dma_start(cos_sb[:], cos_t[:])
            sinm_sb = constp.tile([128, T], bf16)
            nc.sync.dma_start(sinm_sb[:], sinm_t[:])
            if has_qkv_bias:
                bqk_sb = constp.tile([1, 256], bf16)
                nc.sync.dma_start(bqk_sb[:], bqk[:])
                bv_sb = constp.tile([1, 128], bf16)
                nc.sync.dma_start(bv_sb[:], bv[:])
                ones_row = constp.tile([1, 512], bf16)
                nc.gpsimd.memset(ones_row[:], 1.0)
                ones_128 = constp.tile([1, 128], bf16)
                nc.gpsimd.memset(ones_128[:], 1.0)

            KT_res = [resp.tile([128, T], bf16, name=f"KT{b}")
                      for b in range(NB)]
            v4 = []
            for b in range(NB):
                V_res = resp.tile([128, NKT * 130], bf16, name=f"V{b}")
                v4b = V_res[:].rearrange("p (kt h c) -> p kt h c",
                                         kt=NKT, h=2)
                nc.gpsimd.memset(v4b[:, :, :, 64], 1.0)
                v4.append(v4b)

            NCH_ALL = len(chunk_seq)

            def load_chunk(ci):
                b, qc = chunk_seq[ci]
                ch = b * NCHB + qc
                xa = xp.tile([128, CHW], bf16, tag="xa", name="xa")
                nc.sync.dma_start(xa[:], xTt[:, ch * CHW:(ch + 1) * CHW])
                return xa

            def norm_part(st):
                O = st["O"]
                ot_full = otp.tile([128, 512], bf16, tag="ot", name="ot_full")
                for h in range(2):
                    # 1/rowsum = exp(-ln(rowsum)) on ACT (same table set as
                    # the attention Exp)
                    lnv = smp.tile([1, 512], f32, tag="ln", name="lnv")
                    nc.scalar.activation(lnv[:], O[h][64:65, :], AF.Ln)
                    rs_sb = smp.tile([1, 512], f32, tag="rs", name="rs")
                    nc.scalar.activation(rs_sb[:], lnv[:], AF.Exp,
                                         bias=0.0, scale=-1.0)
                    rsb = smp.tile([64, 512], f32, tag="rsb", name="rsb")
                    nc.gpsimd.partition_broadcast(rsb[:], rs_sb[:])
                    nc.vector.tensor_tensor(
                        ot_full[64 * h:64 * h + 64, :],
                        O[h][0:64, :], rsb[:], AL.mult)
                return ot_full

            def rope(ps_f, Q0, dst):
                # RoPE: shuffle swaps the 16-halves of each quadrant
                # (= rotate_half under the host-side PERM64 order)
                shuf = ropep.tile([128, QCH], f32, tag="sh", name="shuf")
                nc.vector.stream_shuffle(shuf[:], ps_f[:], SHUF_MASK)
                t1 = ropep.tile([128, QCH], bf16, tag="t1", name="t1")
                nc.vector.tensor_tensor(
                    t1[:], ps_f[:], cos_sb[:, Q0:Q0 + QCH], AL.mult)
                t2 = ropep.tile([128, QCH], bf16, tag="t2", name="t2")
                nc.vector.tensor_tensor(
                    t2[:], shuf[:], sinm_sb[:, Q0:Q0 + QCH], AL.mult)
                nc.vector.tensor_tensor(dst, t1[:], t2[:], AL.add)

            def x_stage_groups(ci, xa):
                """Emission groups for qkv+v of chunk ci (consumed while the
                previous chunk's attention runs on the PE)."""
                b, qc = chunk_seq[ci]
                Q0 = qc * QCH
                st = {"QT": qtp.tile([128, QCH], bf16, tag="qt", name="QT"),
                      "b": b, "qc": qc, "ci": ci}
                ps_hold = {}

                def qk_half(f, half):
                    def emit():
                        if half == 0:
                            ps_hold[f] = ps_m_p.tile([128, QCH], f32,
                                                     tag="m", name="ps_f")
                        ps_f = ps_hold[f]
                        for dt in range(4 * half, 4 * half + 4):
                            nc.tensor.matmul(
                                ps_f[:],
                                wqk_sb[:, dt * 256 + f * 128:
                                       dt * 256 + (f + 1) * 128],
                                xa[:, dt * 512:(dt + 1) * 512],
                                start=(dt == 0),
                                stop=(dt == DT - 1 and not has_qkv_bias))
                        if half == 1:
                            if has_qkv_bias:
                                nc.tensor.matmul(
                                    ps_f[:],
                                    bqk_sb[:, f * 128:(f + 1) * 128],
                                    ones_row[:], start=False, stop=True)
                            dst = (st["QT"][:] if f == 0
                                   else KT_res[b][:, Q0:Q0 + QCH])
                            rope(ps_f, Q0, dst)
                    return emit

                def v_block(tt, last):
                    def emit():
                        if tt == 0:
                            ps_hold["v"] = ps_m_p.tile([128, QCH], f32,
                                                       tag="m", name="ps_v")
                        ps_v = ps_hold["v"]
                        sl = slice(tt * 128, (tt + 1) * 128)
                        for dt in range(DT):
                            nc.tensor.matmul(
                                ps_v[:, sl],
                                xa[:, dt * 512 + tt * 128:
                                   dt * 512 + (tt + 1) * 128],
                                wv_sb[:, dt * 128:(dt + 1) * 128],
                                start=(dt == 0),
                                stop=(dt == DT - 1 and not has_qkv_bias))
                        if has_qkv_bias:
                            nc.tensor.matmul(ps_v[:, sl], ones_128[:],
                                             bv_sb[:], start=False,
                                             stop=True)
                        if last:
                            kt0 = Q0 // 128
                            nc.vector.tensor_copy(
                                v4[b][:, kt0:kt0 + 4, :, 0:64],
                                ps_v[:].rearrange("p (tt h j) -> p tt h j",
                                                  tt=4, h=2))
                    return emit

                groups = [qk_half(0, 0), qk_half(0, 1),
                          qk_half(1, 0), qk_half(1, 1)]
                groups += [v_block(tt, tt == 3) for tt in range(4)]
                return st, groups

            def proj_groups(st, ot_full):
                """Projection of a finished chunk, split into 4 groups of
                2 matmuls + copies, ending with the single store DMA."""
                ch = st["b"] * NCHB + st["qc"]
                ysb = yp.tile([128, CHW], bf16, tag="y", name="ysb")

                def pair(g):
                    def emit():
                        for oi in (2 * g, 2 * g + 1):
                            ps_y = ps_m_p.tile([128, 512], f32, tag="m",
                                               name="ps_y")
                            nc.tensor.matmul(
                                ps_y[:], pw_sb[:, oi * 128:(oi + 1) * 128],
                                ot_full[:], start=True, stop=True)
                            nc.vector.tensor_copy(
                                ysb[:, oi * 512:(oi + 1) * 512], ps_y[:])
                        if g == 3:
                            nc.sync.dma_start(
                                yTt[:, ch * CHW:(ch + 1) * CHW], ysb[:])
                    return emit
                return [pair(g) for g in range(4)]

            def attention(st, fillers):
                """Attention rounds of chunk st, with filler PE groups (next
                chunk's qkv + prev chunk's proj) interleaved so the PE never
                starves while ACT streams the exps."""
                b, qc = st["b"], st["qc"]
                Q0 = qc * QCH
                QT_cur = st["QT"]
                kt0 = Q0 // 128
                nkb = kt0 + 4
                O = [ps_o_p.tile([65, 512], f32, tag="o", name=f"O{h}")
                     for h in range(2)]
                # diag blocks first (gives affine_select extra slack),
                # then past blocks; S one kb ahead of O hides exp latency
                kb_order = list(range(kt0, nkb)) + list(range(0, kt0))
                pts = {}

                def emit_S(kb):
                    qstart = max(0, 128 * kb - Q0)
                    pth = []
                    for h in range(2):
                        hp = slice(64 * h, 64 * h + 64)
                        ps_sc = ps_s_p.tile([128, 512], f32, tag="s",
                                            name="ps_sc")
                        nc.tensor.matmul(
                            ps_sc[:, qstart:QCH],
                            KT_res[b][hp, kb * 128:(kb + 1) * 128],
                            QT_cur[hp, qstart:QCH],
                            start=True, stop=True)
                        pt = ptp.tile([128, QCH], bf16, tag="pt", name="pt")
                        nc.scalar.activation(pt[:, qstart:QCH],
                                             ps_sc[:, qstart:QCH],
                                             AF.Exp, bias=0.0, scale=0.125)
                        if 128 * kb >= Q0:
                            ds = 128 * kb - Q0
                            nc.gpsimd.affine_select(
                                out=pt[:, ds:ds + 128],
                                in_=pt[:, ds:ds + 128],
                                pattern=[[1, 128]], compare_op=AL.is_ge,
                                fill=0.0, base=0, channel_multiplier=-1)
                        pth.append(pt)
                    pts[kb] = pth

                def emit_O(kb, first, last):
                    qstart = max(0, 128 * kb - Q0)
                    pth = pts.pop(kb)
                    for h in range(2):
                        nc.tensor.matmul(
                            O[h][:, qstart:QCH],
                            v4[b][:, kb, h, :],
                            pth[h][:, qstart:QCH],
                            start=first, stop=last)

                done = 0
                for i, kb in enumerate(kb_order):
                    emit_S(kb)
                    target = (i + 1) * len(fillers) // nkb
                    while done < target:
                        fillers[done]()
                        done += 1
                    if i >= 1:
                        emit_O(kb_order[i - 1], first=(i == 1), last=False)
                emit_O(kb_order[-1], first=(len(kb_order) == 1), last=True)
                while done < len(fillers):
                    fillers[done]()
                    done += 1
                st["O"] = O

            # ---- software pipeline: A(ci) interleaved with X(ci+1)+P(ci-1)
            xa_t = {0: load_chunk(0)}
            if NCH_ALL > 1:
                xa_t[1] = load_chunk(1)
            st_cur, groups0 = x_stage_groups(0, xa_t[0])
            for g in groups0:       # prologue: X(0), nothing to overlap yet
                g()
            prev_st = None
            for ci in range(NCH_ALL):
                if ci + 2 < NCH_ALL:
                    xa_t[ci + 2] = load_chunk(ci + 2)
                if ci + 1 < NCH_ALL:
                    st_next, xg = x_stage_groups(ci + 1, xa_t[ci + 1])
                else:
                    st_next, xg = None, []
                if prev_st is not None:
                    pg = proj_groups(prev_st, norm_part(prev_st))
                else:
                    pg = []
                fillers = xg[:4] + pg + xg[4:]
                attention(st_cur, fillers)
                prev_st = st_cur
                st_cur = st_next
            for g in proj_groups(prev_st, norm_part(prev_st)):
                g()
    nc.compile()
    return nc


# ---------------------------------------------------------------- host side

def _rope_tables(T):
    """cos / sign-baked-sin tables in the PERM64 feature order, bf16."""
    inv_freq = 1.0 / (ROPE_BASE ** (np.arange(0, HD, 2, dtype=np.float64)
                                    / HD))
    pos = np.arange(T, dtype=np.float64)
    ang = np.outer(pos, inv_freq)          # [T, 32]
    cos = np.cos(ang)                      # [T, 32]
    sin = np.sin(ang)
    p = np.arange(128)
    d = np.asarray(PERM64)[p % 64]         # original dim held by partition p
    cos_t = cos[:, d % 32].T               # [128, T]
    sgn = np.where(d < 32, -1.0, 1.0)
    sinm_t = (sin[:, d % 32] * sgn[None, :]).T
    return (np.ascontiguousarray(cos_t).astype(BF16),
            np.ascontiguousarray(sinm_t).astype(BF16))


def make_core_inputs(x, qkv_w, qkv_b, proj_w, NB, T, has_qkv_bias):
    NTOK = NB * T
    NCH = NTOK // QCH
    xf = np.asarray(x).reshape(NTOK, HIDDEN).astype(BF16)
    # xTt[p, ch, dt, c] = x[ch*512+c, dt*128+p]
    xTt = np.ascontiguousarray(
        xf.reshape(NCH, QCH, DT, 128).transpose(3, 0, 2, 1)
    ).reshape(128, NCH * CHW)
    cos_t, sinm_t = _rope_tables(T)
    qkv_w = np.asarray(qkv_w)
    qkv_b = np.asarray(qkv_b)
    perm = np.asarray(PERM64)
    in_maps = []
    for c in range(N_CORES):
        col = HD * H2 * c
        # q/k columns in PERM64 order (per head), v natural
        qcols = np.concatenate([col + perm, col + 64 + perm])
        wq = qkv_w[:, qcols]
        wk = qkv_w[:, HIDDEN + qcols]
        wv = qkv_w[:, 2 * HIDDEN + col:2 * HIDDEN + col + 128]
        wqk_t = np.zeros((128, DT * 256), dtype=np.float32)
        wv_t = np.zeros((128, DT * 128), dtype=np.float32)
        for dt in range(DT):
            rows = slice(dt * 128, (dt + 1) * 128)
            wqk_t[:, dt * 256:dt * 256 + 128] = wq[rows]
            wqk_t[:, dt * 256 + 128:dt * 256 + 256] = wk[rows]
            wv_t[:, dt * 128:(dt + 1) * 128] = wv[rows]
        pwc = np.ascontiguousarray(
            proj_w[col:col + 128, :]).astype(BF16)
        m = {
            "xTt": xTt, "wqk": wqk_t.astype(BF16), "wv": wv_t.astype(BF16),
            "pw": pwc, "cos_t": cos_t, "sinm_t": sinm_t,
        }
        if has_qkv_bias:
            m["bqk"] = np.concatenate(
                [qkv_b[qcols], qkv_b[HIDDEN + qcols]])[None, :].astype(BF16)
            m["bv"] = qkv_b[2 * HIDDEN + col:
                            2 * HIDDEN + col + 128][None, :].astype(BF16)
        in_maps.append(m)
    return in_maps


_PROGRAM_CACHE = {}


def _get_program(NB, T, has_qkv_bias):
    key = (NB, T, has_qkv_bias)
    if key not in _PROGRAM_CACHE:
        _PROGRAM_CACHE[key] = build_program(NB, T, has_qkv_bias)
    return _PROGRAM_CACHE[key]


def run(x, qkv_w, qkv_b, proj_w, proj_b, NB, T, trace=False):
    has_qkv_bias = bool(np.any(np.asarray(qkv_b)))
    nc = _get_program(NB, T, has_qkv_bias)
    in_maps = make_core_inputs(x, qkv_w, qkv_b, proj_w, NB, T, has_qkv_bias)
    res = bass_utils.run_bass_kernel_spmd(
        nc, in_maps, core_ids=list(range(N_CORES)), trace=trace)
    acc = res.results[0]["yTt"].astype(np.float32)
    for c in range(1, N_CORES):
        acc += res.results[c]["yTt"].astype(np.float32)
    NTOK = NB * T
    NCH = NTOK // QCH
    # y[ch*512+c, oi*128+p] = acc[p, ch, oi, c]
    y = acc.reshape(128, NCH, DT, QCH).transpose(1, 3, 2, 0).reshape(
        NTOK, HIDDEN)
    out = (y.reshape(NB, T, HIDDEN)
           + np.asarray(proj_b)[None, None, :].astype(np.float32))
    return out, res


def kernel(x, qkv_w, qkv_b, proj_w, proj_b):
    x = np.asarray(x)
    B, L, D = x.shape
    out, _ = run(x, np.asarray(qkv_w), np.asarray(qkv_b),
                 np.asarray(proj_w), np.asarray(proj_b), NB=B, T=L)
    return out.astype(np.float32)


# revision 17
# speedup vs baseline: 1.7371x; 1.0941x over previous
"""Trainium2 Bass kernel for nn_CausalSelfAttention (B=4, L=2048, D=1024, H=16).

Sharding: 2 heads per core (tensor parallel) x 8 cores; every core sees all
B*L tokens.  Each core computes q/k/v for its 2 heads, runs causal attention,
and emits a partial projection y_c = proj_w[rows_c].T @ O_c of shape
[B*L, D] (chunk-tiled) in bf16.  The host sums the 8 partials in fp32 and
adds proj_b.

Key layout choices (all bf16 on device, fp32 accumulation in PSUM):
  - x is pre-transposed AND chunk-tiled on the host (xTt[p, ch, dt, c] =
    x[ch*512+c, dt*128+p]), so each 512-token chunk is ONE contiguous-line
    DMA (8KB/partition) and no PE transposes are needed.  Output is tiled
    the same way: one 8KB-line store per chunk.
  - RoPE without matmuls: the head-dim feature order is permuted on the host
    (pairs (j, j+32) land in the same 32-partition quadrant), so rotate_half
    is a single DVE stream_shuffle; the sign is baked into the sin table.
    rope(q) = q*cos + shuffle(q)*sin_signed.
  - Attention S^T/P^T/O^T per head with [V|1] ones-column rowsum trick;
    1/rowsum via DVE reciprocal_approx_fast + stride-0 broadcast AP;
    causal diag via gpsimd affine_select.
  - Chunks are BATCH-INTERLEAVED ((b0,qc0),(b1,qc0),...,(b0,qc1),...) with
    per-batch K/V residents, so the small-attention batch-start chunks don't
    stall the pipe; normalize+projection of chunk i-1 are emitted inside
    chunk i, and S(kb+1) is emitted before O(kb) to hide exp latency.
"""

import numpy as np
import ml_dtypes

import concourse.bass as bass  # noqa: F401
import concourse.tile as tile
from concourse import mybir, bacc
from concourse import bass_utils

f32 = mybir.dt.float32
bf16 = mybir.dt.bfloat16
AL = mybir.AluOpType
AF = mybir.ActivationFunctionType

BF16 = ml_dtypes.bfloat16


class _Bacc(bacc.Bacc):
    """Bacc that pins all activations to the one table set holding exp, so
    ACT_TABLE_LOADs (~2.7us each) happen once."""

    def insert_act_table_loads(self):
        import bass_rust as _bass_rust
        from concourse.hw_specs import get_activation_tables

        has_activation = any(
            isinstance(i, mybir.InstActivation)
            for bb in self.main_func.blocks
            for i in bb.instructions
        )
        if not has_activation:
            return
        tables = [
            (k, v if k == "natural_log_exp_and_others" else set())
            for k, v in get_activation_tables(self.m.arch).items()
        ]
        _bass_rust.insert_act_table_loads(self, tables)


HIDDEN = 1024
HEADS = 16
HD = 64
ROPE_BASE = 10000.0
N_CORES = 8
H2 = 2            # heads per core
QCH = 512         # token chunk = attention q granule
DT = HIDDEN // 128  # 8 d tiles
CHW = DT * QCH    # flattened chunk width in the tiled x / y layouts

# within-head feature permutation: rope pairs (j, j+32) share a quadrant
PERM64 = list(range(0, 16)) + list(range(32, 48)) + \
    list(range(16, 32)) + list(range(48, 64))
# stream_shuffle mask: swap 16-halves within each 32-partition quadrant
SHUF_MASK = [(i + 16) % 32 for i in range(32)]


def build_program(NB, T, has_qkv_bias):
    """Build the per-core Bass program: NB batches of T tokens each."""
    assert T % QCH == 0
    NCHB = T // QCH          # chunks per batch
    NCH = NB * NCHB          # total chunks
    NKT = T // 128           # k tiles per batch
    nc = _Bacc("TRN2", target_bir_lowering=False, debug=False,
               num_devices=N_CORES)

    xTt = nc.dram_tensor("xTt", [128, NCH * CHW], bf16,
                         kind="ExternalInput").ap()
    wqk = nc.dram_tensor("wqk", [128, DT * 256], bf16,
                         kind="ExternalInput").ap()
    wv = nc.dram_tensor("wv", [128, DT * 128], bf16,
                        kind="ExternalInput").ap()
    pw = nc.dram_tensor("pw", [128, HIDDEN], bf16, kind="ExternalInput").ap()
    cos_t = nc.dram_tensor("cos_t", [128, T], bf16, kind="ExternalInput").ap()
    sinm_t = nc.dram_tensor("sinm_t", [128, T], bf16,
                            kind="ExternalInput").ap()
    if has_qkv_bias:
        bqk = nc.dram_tensor("bqk", [1, 256], bf16, kind="ExternalInput").ap()
        bv = nc.dram_tensor("bv", [1, 128], bf16, kind="ExternalInput").ap()
    yTt = nc.dram_tensor("yTt", [128, NCH * CHW], bf16,
                         kind="ExternalOutput").ap()

    # batch-interleaved chunk order; ch = b * NCHB + qc indexes the tiles
    chunk_seq = [(b, qc) for qc in range(NCHB) for b in range(NB)]

    with tile.TileContext(nc) as tc:
        with tc.tile_pool(name="const", bufs=1) as constp, \
             tc.tile_pool(name="resident", bufs=1) as resp, \
             tc.tile_pool(name="xload", bufs=3) as xp, \
             tc.tile_pool(name="rope", bufs=2) as ropep, \
             tc.tile_pool(name="qtcur", bufs=2) as qtp, \
             tc.tile_pool(name="pt", bufs=4) as ptp, \
             tc.tile_pool(name="ot", bufs=2) as otp, \
             tc.tile_pool(name="ysb", bufs=2) as yp, \
             tc.tile_pool(name="small", bufs=4) as smp, \
             tc.tile_pool(name="ps_s", bufs=2, space="PSUM") as ps_s_p, \
             tc.tile_pool(name="ps_o", bufs=2, space="PSUM") as ps_o_p, \
             tc.tile_pool(name="ps_m", bufs=2, space="PSUM") as ps_m_p:

            # ---- constants / residents ----
            wqk_sb = constp.tile([128, DT * 256], bf16)
            nc.sync.dma_start(wqk_sb[:], wqk[:])
            wv_sb = constp.tile([128, DT * 128], bf16)
            nc.sync.dma_start(wv_sb[:], wv[:])
            pw_sb = constp.tile([128, HIDDEN], bf16)
            nc.sync.dma_start(pw_sb[:], pw[:])
            cos_sb = constp.tile([128, T], bf16)
            nc.sync.dma_start(cos_sb[:], cos_t[:])
            sinm_sb = constp.tile([128, T], bf16)
            nc.sync.dma_start(sinm_sb[:], sinm_t[:])
            if has_qkv_bias:
                bqk_sb = constp.tile([1, 256], bf16)
                nc.sync.dma_start(bqk_sb[:], bqk[:])
                bv_sb = constp.tile([1, 128], bf16)
                nc.sync.dma_start(bv_sb[:], bv[:])
                ones_row = constp.tile([1, 512], bf16)
                nc.gpsimd.memset(ones_row[:], 1.0)
                ones_128 = constp.tile([1, 128], bf16)
                nc.gpsimd.memset(ones_128[:], 1.0)

            # KT zero-padded per head to a full K=128 contraction (S matmuls
            # with K=64 stream at ~1.5 cycles/row; padded ones hit ~1.0):
            # kt2[b][:, h, t] holds head h's K rows in its own 64-partition
            # band, zeros elsewhere, so lhsT can use all 128 partitions
            # against the 2-head QT moving operand.
            kt2 = []
            for b in range(NB):
                KT = resp.tile([128, 2 * T], bf16, name=f"KT{b}")
                k2 = KT[:].rearrange("p (h t) -> p h t", h=2)
                eng = nc.vector if b % 2 == 0 else nc.gpsimd
                eng.memset(k2[64:128, 0, :], 0.0)
                eng.memset(k2[0:64, 1, :], 0.0)
                kt2.append(k2)
            v4 = []
            for b in range(NB):
                V_res = resp.tile([128, NKT * 130], bf16, name=f"V{b}")
                v4b = V_res[:].rearrange("p (kt h c) -> p kt h c",
                                         kt=NKT, h=2)
                nc.gpsimd.memset(v4b[:, :, :, 64], 1.0)
                v4.append(v4b)

            NCH_ALL = len(chunk_seq)

            def load_chunk(ci):
                b, qc = chunk_seq[ci]
                ch = b * NCHB + qc
                xa = xp.tile([128, CHW], bf16, tag="xa", name="xa")
                nc.sync.dma_start(xa[:], xTt[:, ch * CHW:(ch + 1) * CHW])
                return xa

            def norm_part(st):
                O = st["O"]
                ot_full = otp.tile([128, 512], bf16, tag="ot", name="ot_full")
                for h in range(2):
                    # 1/rowsum = exp(-ln(rowsum)) on ACT (same table set as
                    # the attention Exp)
                    lnv = smp.tile([1, 512], f32, tag="ln", name="lnv")
                    nc.scalar.activation(lnv[:], O[h][64:65, :], AF.Ln)
                    rs_sb = smp.tile([1, 512], f32, tag="rs", name="rs")
                    nc.scalar.activation(rs_sb[:], lnv[:], AF.Exp,
                                         bias=0.0, scale=-1.0)
                    rsb = smp.tile([64, 512], f32, tag="rsb", name="rsb")
                    nc.gpsimd.partition_broadcast(rsb[:], rs_sb[:])
                    nc.vector.tensor_tensor(
                        ot_full[64 * h:64 * h + 64, :],
                        O[h][0:64, :], rsb[:], AL.mult)
                return ot_full

            def rope(ps_f, Q0, dsts):
                # RoPE: shuffle swaps the 16-halves of each quadrant
                # (= rotate_half under the host-side PERM64 order)
                shuf = ropep.tile([128, QCH], f32, tag="sh", name="shuf")
                nc.vector.stream_shuffle(shuf[:], ps_f[:], SHUF_MASK)
                t1 = ropep.tile([128, QCH], bf16, tag="t1", name="t1")
                nc.vector.tensor_tensor(
                    t1[:], ps_f[:], cos_sb[:, Q0:Q0 + QCH], AL.mult)
                t2 = ropep.tile([128, QCH], bf16, tag="t2", name="t2")
                nc.vector.tensor_tensor(
                    t2[:], shuf[:], sinm_sb[:, Q0:Q0 + QCH], AL.mult)
                for rows, dst in dsts:
                    nc.vector.tensor_tensor(
                        dst, t1[rows, :], t2[rows, :], AL.add)

            def x_stage_groups(ci, xa):
                """Emission groups for qkv+v of chunk ci (consumed while the
                previous chunk's attention runs on the PE)."""
                b, qc = chunk_seq[ci]
                Q0 = qc * QCH
                st = {"QT": qtp.tile([128, QCH], bf16, tag="qt", name="QT"),
                      "b": b, "qc": qc, "ci": ci}
                ps_hold = {}

                def qk_half(f, half):
                    def emit():
                        if half == 0:
                            ps_hold[f] = ps_m_p.tile([128, QCH], f32,
                                                     tag="m", name="ps_f")
                        ps_f = ps_hold[f]
                        for dt in range(4 * half, 4 * half + 4):
                            nc.tensor.matmul(
                                ps_f[:],
                                wqk_sb[:, dt * 256 + f * 128:
                                       dt * 256 + (f + 1) * 128],
                                xa[:, dt * 512:(dt + 1) * 512],
                                start=(dt == 0),
                                stop=(dt == DT - 1 and not has_qkv_bias))
                        if half == 1:
                            if has_qkv_bias:
                                nc.tensor.matmul(
                                    ps_f[:],
                                    bqk_sb[:, f * 128:(f + 1) * 128],
                                    ones_row[:], start=False, stop=True)
                            if f == 0:
                                dsts = [(slice(0, 128), st["QT"][:])]
                            else:
                                dsts = [
                                    (slice(0, 64),
                                     kt2[b][0:64, 0, Q0:Q0 + QCH]),
                                    (slice(64, 128),
                                     kt2[b][64:128, 1, Q0:Q0 + QCH]),
                                ]
                            rope(ps_f, Q0, dsts)
                    return emit

                def v_block(tt, last):
                    def emit():
                        if tt == 0:
                            ps_hold["v"] = ps_m_p.tile([128, QCH], f32,
                                                       tag="m", name="ps_v")
                        ps_v = ps_hold["v"]
                        sl = slice(tt * 128, (tt + 1) * 128)
                        for dt in range(DT):
                            nc.tensor.matmul(
                                ps_v[:, sl],
                                xa[:, dt * 512 + tt * 128:
                                   dt * 512 + (tt + 1) * 128],
                                wv_sb[:, dt * 128:(dt + 1) * 128],
                                start=(dt == 0),
                                stop=(dt == DT - 1 and not has_qkv_bias))
                        if has_qkv_bias:
                            nc.tensor.matmul(ps_v[:, sl], ones_128[:],
                                             bv_sb[:], start=False,
                                             stop=True)
                        if last:
                            kt0 = Q0 // 128
                            nc.vector.tensor_copy(
                                v4[b][:, kt0:kt0 + 4, :, 0:64],
                                ps_v[:].rearrange("p (tt h j) -> p tt h j",
                                                  tt=4, h=2))
                    return emit

                groups = [qk_half(0, 0), qk_half(0, 1),
                          qk_half(1, 0), qk_half(1, 1)]
                groups += [v_block(tt, tt == 3) for tt in range(4)]
                return st, groups

            def proj_groups(st, ot_full):
                """Projection of a finished chunk, split into 4 groups of
                2 matmuls + copies, ending with the single store DMA."""
                ch = st["b"] * NCHB + st["qc"]
                ysb = yp.tile([128, CHW], bf16, tag="y", name="ysb")

                def pair(g):
                    def emit():
                        for oi in (2 * g, 2 * g + 1):
                            ps_y = ps_m_p.tile([128, 512], f32, tag="m",
                                               name="ps_y")
                            nc.tensor.matmul(
                                ps_y[:], pw_sb[:, oi * 128:(oi + 1) * 128],
                                ot_full[:], start=True, stop=True)
                            nc.vector.tensor_copy(
                                ysb[:, oi * 512:(oi + 1) * 512], ps_y[:])
                        if g == 3:
                            nc.sync.dma_start(
                                yTt[:, ch * CHW:(ch + 1) * CHW], ysb[:])
                    return emit
                return [pair(g) for g in range(4)]

            def attention(st, fillers):
                """Attention rounds of chunk st, with filler PE groups (next
                chunk's qkv + prev chunk's proj) interleaved so the PE never
                starves while ACT streams the exps."""
                b, qc = st["b"], st["qc"]
                Q0 = qc * QCH
                QT_cur = st["QT"]
                kt0 = Q0 // 128
                nkb = kt0 + 4
                O = [ps_o_p.tile([65, 512], f32, tag="o", name=f"O{h}")
                     for h in range(2)]
                # diag blocks first (gives affine_select extra slack),
                # then past blocks; S one kb ahead of O hides exp latency
                kb_order = list(range(kt0, nkb)) + list(range(0, kt0))
                pts = {}

                def emit_S(kb):
                    qstart = max(0, 128 * kb - Q0)
                    # both heads' scores in one 2-bank psum tile; K=128
                    # zero-padded KT keeps the PE at 1 cycle/row, and a
                    # single exp over [p, 2, q] halves ACT op overhead
                    ps_sc = ps_s_p.tile([128, 2 * QCH], f32, tag="s",
                                        name="ps_sc")
                    for h in range(2):
                        nc.tensor.matmul(
                            ps_sc[:, QCH * h + qstart:QCH * h + QCH],
                            kt2[b][:, h, kb * 128:(kb + 1) * 128],
                            QT_cur[:, qstart:QCH],
                            start=True, stop=True)
                    pt = ptp.tile([128, 2 * QCH], bf16, tag="pt", name="pt")
                    sc4 = ps_sc[:].rearrange("p (h q) -> p h q", h=2)
                    pt4 = pt[:].rearrange("p (h q) -> p h q", h=2)
                    nc.scalar.activation(pt4[:, :, qstart:QCH],
                                         sc4[:, :, qstart:QCH],
                                         AF.Exp, bias=0.0, scale=0.125)
                    if 128 * kb >= Q0:
                        ds = 128 * kb - Q0
                        for h in range(2):
                            nc.gpsimd.affine_select(
                                out=pt[:, QCH * h + ds:QCH * h + ds + 128],
                                in_=pt[:, QCH * h + ds:QCH * h + ds + 128],
                                pattern=[[1, 128]], compare_op=AL.is_ge,
                                fill=0.0, base=0, channel_multiplier=-1)
                    pts[kb] = pt

                def emit_O(kb, first, last):
                    qstart = max(0, 128 * kb - Q0)
                    pt = pts.pop(kb)
                    for h in range(2):
                        nc.tensor.matmul(
                            O[h][:, qstart:QCH],
                            v4[b][:, kb, h, :],
                            pt[:, QCH * h + qstart:QCH * h + QCH],
                            start=first, stop=last)

                done = 0
                for i, kb in enumerate(kb_order):
                    emit_S(kb)
                    target = (i + 1) * len(fillers) // nkb
                    while done < target:
                        fillers[done]()
                        done += 1
                    if i >= 1:
                        emit_O(kb_order[i - 1], first=(i == 1), last=False)
                emit_O(kb_order[-1], first=(len(kb_order) == 1), last=True)
                while done < len(fillers):
                    fillers[done]()
                    done += 1
                st["O"] = O

            # ---- software pipeline: A(ci) interleaved with X(ci+1)+P(ci-1)
            xa_t = {0: load_chunk(0)}
            if NCH_ALL > 1:
                xa_t[1] = load_chunk(1)
            st_cur, groups0 = x_stage_groups(0, xa_t[0])
            for g in groups0:       # prologue: X(0), nothing to overlap yet
                g()
            prev_st = None
            for ci in range(NCH_ALL):
                if ci + 2 < NCH_ALL:
                    xa_t[ci + 2] = load_chunk(ci + 2)
                if ci + 1 < NCH_ALL:
                    st_next, xg = x_stage_groups(ci + 1, xa_t[ci + 1])
                else:
                    st_next, xg = None, []
                if prev_st is not None:
                    pg = proj_groups(prev_st, norm_part(prev_st))
                else:
                    pg = []
                fillers = xg[:4] + pg + xg[4:]
                attention(st_cur, fillers)
                prev_st = st_cur
                st_cur = st_next
            for g in proj_groups(prev_st, norm_part(prev_st)):
                g()
    nc.compile()
    return nc


# ---------------------------------------------------------------- host side

def _rope_tables(T):
    """cos / sign-baked-sin tables in the PERM64 feature order, bf16."""
    inv_freq = 1.0 / (ROPE_BASE ** (np.arange(0, HD, 2, dtype=np.float64)
                                    / HD))
    pos = np.arange(T, dtype=np.float64)
    ang = np.outer(pos, inv_freq)          # [T, 32]
    cos = np.cos(ang)                      # [T, 32]
    sin = np.sin(ang)
    p = np.arange(128)
    d = np.asarray(PERM64)[p % 64]         # original dim held by partition p
    cos_t = cos[:, d % 32].T               # [128, T]
    sgn = np.where(d < 32, -1.0, 1.0)
    sinm_t = (sin[:, d % 32] * sgn[None, :]).T
    return (np.ascontiguousarray(cos_t).astype(BF16),
            np.ascontiguousarray(sinm_t).astype(BF16))


def make_core_inputs(x, qkv_w, qkv_b, proj_w, NB, T, has_qkv_bias):
    NTOK = NB * T
    NCH = NTOK // QCH
    xf = np.asarray(x).reshape(NTOK, HIDDEN).astype(BF16)
    # xTt[p, ch, dt, c] = x[ch*512+c, dt*128+p]
    xTt = np.ascontiguousarray(
        xf.reshape(NCH, QCH, DT, 128).transpose(3, 0, 2, 1)
    ).reshape(128, NCH * CHW)
    cos_t, sinm_t = _rope_tables(T)
    qkv_w = np.asarray(qkv_w)
    qkv_b = np.asarray(qkv_b)
    perm = np.asarray(PERM64)
    in_maps = []
    for c in range(N_CORES):
        col = HD * H2 * c
        # q/k columns in PERM64 order (per head), v natural
        qcols = np.concatenate([col + perm, col + 64 + perm])
        wq = qkv_w[:, qcols]
        wk = qkv_w[:, HIDDEN + qcols]
        wv = qkv_w[:, 2 * HIDDEN + col:2 * HIDDEN + col + 128]
        wqk_t = np.zeros((128, DT * 256), dtype=np.float32)
        wv_t = np.zeros((128, DT * 128), dtype=np.float32)
        for dt in range(DT):
            rows = slice(dt * 128, (dt + 1) * 128)
            wqk_t[:, dt * 256:dt * 256 + 128] = wq[rows]
            wqk_t[:, dt * 256 + 128:dt * 256 + 256] = wk[rows]
            wv_t[:, dt * 128:(dt + 1) * 128] = wv[rows]
        pwc = np.ascontiguousarray(
            proj_w[col:col + 128, :]).astype(BF16)
        m = {
            "xTt": xTt, "wqk": wqk_t.astype(BF16), "wv": wv_t.astype(BF16),
            "pw": pwc, "cos_t": cos_t, "sinm_t": sinm_t,
        }
        if has_qkv_bias:
            m["bqk"] = np.concatenate(
                [qkv_b[qcols], qkv_b[HIDDEN + qcols]])[None, :].astype(BF16)
            m["bv"] = qkv_b[2 * HIDDEN + col:
                            2 * HIDDEN + col + 128][None, :].astype(BF16)
        in_maps.append(m)
    return in_maps


_PROGRAM_CACHE = {}


def _get_program(NB, T, has_qkv_bias):
    key = (NB, T, has_qkv_bias)
    if key not in _PROGRAM_CACHE:
        _PROGRAM_CACHE[key] = build_program(NB, T, has_qkv_bias)
    return _PROGRAM_CACHE[key]


def run(x, qkv_w, qkv_b, proj_w, proj_b, NB, T, trace=False):
    has_qkv_bias = bool(np.any(np.asarray(qkv_b)))
    nc = _get_program(NB, T, has_qkv_bias)
    in_maps = make_core_inputs(x, qkv_w, qkv_b, proj_w, NB, T, has_qkv_bias)
    res = bass_utils.run_bass_kernel_spmd(
        nc, in_maps, core_ids=list(range(N_CORES)), trace=trace)
    acc = res.results[0]["yTt"].astype(np.float32)
    for c in range(1, N_CORES):
        acc += res.results[c]["yTt"].astype(np.float32)
    NTOK = NB * T
    NCH = NTOK // QCH
    # y[ch*512+c, oi*128+p] = acc[p, ch, oi, c]
    y = acc.reshape(128, NCH, DT, QCH).transpose(1, 3, 2, 0).reshape(
        NTOK, HIDDEN)
    out = (y.reshape(NB, T, HIDDEN)
           + np.asarray(proj_b)[None, None, :].astype(np.float32))
    return out, res


def kernel(x, qkv_w, qkv_b, proj_w, proj_b):
    x = np.asarray(x)
    B, L, D = x.shape
    out, _ = run(x, np.asarray(qkv_w), np.asarray(qkv_b),
                 np.asarray(proj_w), np.asarray(proj_b), NB=B, T=L)
    return out.astype(np.float32)


# revision 25
# speedup vs baseline: 1.7670x; 1.0172x over previous
"""Trainium2 Bass kernel for nn_CausalSelfAttention (B=4, L=2048, D=1024, H=16).

Sharding: 2 heads per core (tensor parallel) x 8 cores; every core sees all
B*L tokens.  Each core computes q/k/v for its 2 heads, runs causal attention,
and emits a partial projection y_c = proj_w[rows_c].T @ O_c of shape
[B*L, D] (chunk-tiled) in bf16.  The host sums the 8 partials in fp32 and
adds proj_b.

Key layout choices (all bf16 on device, fp32 accumulation in PSUM):
  - x is pre-transposed AND chunk-tiled on the host (xTt[p, ch, dt, c] =
    x[ch*512+c, dt*128+p]), so each 512-token chunk is ONE contiguous-line
    DMA (8KB/partition) and no PE transposes are needed.  Output is tiled
    the same way: one 8KB-line store per chunk.
  - RoPE without matmuls: the head-dim feature order is permuted on the host
    (pairs (j, j+32) land in the same 32-partition quadrant), so rotate_half
    is a single DVE stream_shuffle; the sign is baked into the sin table.
    rope(q) = q*cos + shuffle(q)*sin_signed.
  - Attention S^T/P^T/O^T per head with [V|1] ones-column rowsum trick;
    1/rowsum via DVE reciprocal_approx_fast + stride-0 broadcast AP;
    causal diag via gpsimd affine_select.
  - Chunks are BATCH-INTERLEAVED ((b0,qc0),(b1,qc0),...,(b0,qc1),...) with
    per-batch K/V residents, so the small-attention batch-start chunks don't
    stall the pipe; normalize+projection of chunk i-1 are emitted inside
    chunk i, and S(kb+1) is emitted before O(kb) to hide exp latency.
"""

import numpy as np
import ml_dtypes

import concourse.bass as bass  # noqa: F401
import concourse.tile as tile
from concourse import mybir, bacc
from concourse import bass_utils

f32 = mybir.dt.float32
bf16 = mybir.dt.bfloat16
AL = mybir.AluOpType
AF = mybir.ActivationFunctionType

BF16 = ml_dtypes.bfloat16


class _Bacc(bacc.Bacc):
    """Bacc that pins all activations to the one table set holding exp, so
    ACT_TABLE_LOADs (~2.7us each) happen once."""

    def insert_act_table_loads(self):
        import bass_rust as _bass_rust
        from concourse.hw_specs import get_activation_tables

        has_activation = any(
            isinstance(i, mybir.InstActivation)
            for bb in self.main_func.blocks
            for i in bb.instructions
        )
        if not has_activation:
            return
        tables = [
            (k, v if k == "natural_log_exp_and_others" else set())
            for k, v in get_activation_tables(self.m.arch).items()
        ]
        _bass_rust.insert_act_table_loads(self, tables)


HIDDEN = 1024
HEADS = 16
HD = 64
ROPE_BASE = 10000.0
N_CORES = 8
H2 = 2            # heads per core
QCH = 512         # token chunk = attention q granule
DT = HIDDEN // 128  # 8 d tiles
CHW = DT * QCH    # flattened chunk width in the tiled x / y layouts

# within-head feature permutation: rope pairs (j, j+32) share a quadrant
PERM64 = list(range(0, 16)) + list(range(32, 48)) + \
    list(range(16, 32)) + list(range(48, 64))
# stream_shuffle mask: swap 16-halves within each 32-partition quadrant
SHUF_MASK = [(i + 16) % 32 for i in range(32)]


def build_program(NB, T, has_qkv_bias):
    """Build the per-core Bass program: NB batches of T tokens each."""
    assert T % QCH == 0
    NCHB = T // QCH          # chunks per batch
    NCH = NB * NCHB          # total chunks
    NKT = T // 128           # k tiles per batch
    nc = _Bacc("TRN2", target_bir_lowering=False, debug=False,
               num_devices=N_CORES)

    xTt = nc.dram_tensor("xTt", [128, NCH * CHW], bf16,
                         kind="ExternalInput").ap()
    wqk = nc.dram_tensor("wqk", [128, DT * 256], bf16,
                         kind="ExternalInput").ap()
    wv = nc.dram_tensor("wv", [128, DT * 128], bf16,
                        kind="ExternalInput").ap()
    pw = nc.dram_tensor("pw", [128, HIDDEN], bf16, kind="ExternalInput").ap()
    cos_t = nc.dram_tensor("cos_t", [128, T], bf16, kind="ExternalInput").ap()
    sinm_t = nc.dram_tensor("sinm_t", [128, T], bf16,
                            kind="ExternalInput").ap()
    if has_qkv_bias:
        bqk = nc.dram_tensor("bqk", [1, 256], bf16, kind="ExternalInput").ap()
        bv = nc.dram_tensor("bv", [1, 128], bf16, kind="ExternalInput").ap()
    ktz = nc.dram_tensor("ktz", [128, 2 * T], bf16, kind="ExternalInput").ap()
    yTt = nc.dram_tensor("yTt", [128, NCH * CHW], bf16,
                         kind="ExternalOutput").ap()

    # batch-interleaved chunk order; ch = b * NCHB + qc indexes the tiles
    chunk_seq = [(b, qc) for qc in range(NCHB) for b in range(NB)]

    with tile.TileContext(nc) as tc:
        with tc.tile_pool(name="const", bufs=1) as constp, \
             tc.tile_pool(name="resident", bufs=1) as resp, \
             tc.tile_pool(name="xload", bufs=3) as xp, \
             tc.tile_pool(name="rope", bufs=2) as ropep, \
             tc.tile_pool(name="qtcur", bufs=2) as qtp, \
             tc.tile_pool(name="pt", bufs=4) as ptp, \
             tc.tile_pool(name="ot", bufs=2) as otp, \
             tc.tile_pool(name="ysb", bufs=2) as yp, \
             tc.tile_pool(name="small", bufs=4) as smp, \
             tc.tile_pool(name="ps_s", bufs=2, space="PSUM") as ps_s_p, \
             tc.tile_pool(name="ps_o", bufs=2, space="PSUM") as ps_o_p, \
             tc.tile_pool(name="ps_m", bufs=2, space="PSUM") as ps_m_p:

            # ---- constants / residents ----
            wqk_sb = constp.tile([128, DT * 256], bf16)
            nc.sync.dma_start(wqk_sb[:], wqk[:])
            wv_sb = constp.tile([128, DT * 128], bf16)
            nc.sync.dma_start(wv_sb[:], wv[:])
            pw_sb = constp.tile([128, HIDDEN], bf16)
            nc.sync.dma_start(pw_sb[:], pw[:])
            cos_sb = constp.tile([128, T], bf16)
            nc.sync.dma_start(cos_sb[:], cos_t[:])
            sinm_sb = constp.tile([128, T], bf16)
            nc.sync.dma_start(sinm_sb[:], sinm_t[:])
            if has_qkv_bias:
                bqk_sb = constp.tile([1, 256], bf16)
                nc.sync.dma_start(bqk_sb[:], bqk[:])
                bv_sb = constp.tile([1, 128], bf16)
                nc.sync.dma_start(bv_sb[:], bv[:])
                ones_row = constp.tile([1, 512], bf16)
                nc.gpsimd.memset(ones_row[:], 1.0)
                ones_128 = constp.tile([1, 128], bf16)
                nc.gpsimd.memset(ones_128[:], 1.0)

            # KT zero-padded per head to a full K=128 contraction (S matmuls
            # with K=64 stream at ~1.5 cycles/row; padded ones hit ~1.0):
            # kt2[b][:, h, t] holds head h's K rows in its own 64-partition
            # band, zeros elsewhere, so lhsT can use all 128 partitions
            # against the 2-head QT moving operand.
            # batch 0's pads are needed within a few us: memset split across
            # two engines; batches 1-3 zero-fill via DMA (engines stay free),
            # with the triggers emitted after the first x loads
            kt_zero_dmas = []
            kt2 = []
            for b in range(NB):
                KT = resp.tile([128, 2 * T], bf16, name=f"KT{b}")
                k2 = KT[:].rearrange("p (h t) -> p h t", h=2)
                if b == 0:
                    nc.vector.memset(k2[64:128, 0, :], 0.0)
                    nc.gpsimd.memset(k2[0:64, 1, :], 0.0)
                else:
                    kt_zero_dmas.append(KT)
                kt2.append(k2)
            v4 = []
            for b in range(NB):
                V_res = resp.tile([128, NKT * 130], bf16, name=f"V{b}")
                v4b = V_res[:].rearrange("p (kt h c) -> p kt h c",
                                         kt=NKT, h=2)
                nc.gpsimd.memset(v4b[:, :, :, 64], 1.0)
                v4.append(v4b)

            NCH_ALL = len(chunk_seq)

            def load_chunk(ci):
                b, qc = chunk_seq[ci]
                ch = b * NCHB + qc
                xa = xp.tile([128, CHW], bf16, tag="xa", name="xa")
                nc.sync.dma_start(xa[:], xTt[:, ch * CHW:(ch + 1) * CHW])
                return xa

            def norm_part(st):
                O = st["O"]
                ot_full = otp.tile([128, 512], bf16, tag="ot", name="ot_full")
                for h in range(2):
                    # 1/rowsum = exp(-ln(rowsum)) on ACT (same table set as
                    # the attention Exp)
                    lnv = smp.tile([1, 512], f32, tag="ln", name="lnv")
                    nc.scalar.activation(lnv[:], O[h][64:65, :], AF.Ln)
                    rs_sb = smp.tile([1, 512], f32, tag="rs", name="rs")
                    nc.scalar.activation(rs_sb[:], lnv[:], AF.Exp,
                                         bias=0.0, scale=-1.0)
                    rsb = smp.tile([64, 512], f32, tag="rsb", name="rsb")
                    nc.gpsimd.partition_broadcast(rsb[:], rs_sb[:])
                    nc.vector.tensor_tensor(
                        ot_full[64 * h:64 * h + 64, :],
                        O[h][0:64, :], rsb[:], AL.mult)
                return ot_full

            def rope(ps_f, Q0, dsts):
                # RoPE: shuffle swaps the 16-halves of each quadrant
                # (= rotate_half under the host-side PERM64 order)
                shuf = ropep.tile([128, QCH], f32, tag="sh", name="shuf")
                nc.vector.stream_shuffle(shuf[:], ps_f[:], SHUF_MASK)
                t1 = ropep.tile([128, QCH], bf16, tag="t1", name="t1")
                nc.vector.tensor_tensor(
                    t1[:], ps_f[:], cos_sb[:, Q0:Q0 + QCH], AL.mult)
                t2 = ropep.tile([128, QCH], bf16, tag="t2", name="t2")
                nc.vector.tensor_tensor(
                    t2[:], shuf[:], sinm_sb[:, Q0:Q0 + QCH], AL.mult)
                for rows, dst in dsts:
                    nc.vector.tensor_tensor(
                        dst, t1[rows, :], t2[rows, :], AL.add)

            def x_stage_groups(ci, xa):
                """Emission groups for qkv+v of chunk ci (consumed while the
                previous chunk's attention runs on the PE)."""
                b, qc = chunk_seq[ci]
                Q0 = qc * QCH
                st = {"QT": qtp.tile([128, QCH], bf16, tag="qt", name="QT"),
                      "b": b, "qc": qc, "ci": ci}
                ps_hold = {}

                def qk_half(f, half):
                    def emit():
                        if half == 0:
                            ps_hold[f] = ps_m_p.tile([128, QCH], f32,
                                                     tag="m", name="ps_f")
                        ps_f = ps_hold[f]
                        for dt in range(4 * half, 4 * half + 4):
                            nc.tensor.matmul(
                                ps_f[:],
                                wqk_sb[:, dt * 256 + f * 128:
                                       dt * 256 + (f + 1) * 128],
                                xa[:, dt * 512:(dt + 1) * 512],
                                start=(dt == 0),
                                stop=(dt == DT - 1 and not has_qkv_bias))
                        if half == 1:
                            if has_qkv_bias:
                                nc.tensor.matmul(
                                    ps_f[:],
                                    bqk_sb[:, f * 128:(f + 1) * 128],
                                    ones_row[:], start=False, stop=True)
                            if f == 0:
                                dsts = [(slice(0, 128), st["QT"][:])]
                            else:
                                dsts = [
                                    (slice(0, 64),
                                     kt2[b][0:64, 0, Q0:Q0 + QCH]),
                                    (slice(64, 128),
                                     kt2[b][64:128, 1, Q0:Q0 + QCH]),
                                ]
                            rope(ps_f, Q0, dsts)
                    return emit

                def v_block(tt, last):
                    def emit():
                        if tt == 0:
                            ps_hold["v"] = ps_m_p.tile([128, QCH], f32,
                                                       tag="m", name="ps_v")
                        ps_v = ps_hold["v"]
                        sl = slice(tt * 128, (tt + 1) * 128)
                        for dt in range(DT):
                            nc.tensor.matmul(
                                ps_v[:, sl],
                                xa[:, dt * 512 + tt * 128:
                                   dt * 512 + (tt + 1) * 128],
                                wv_sb[:, dt * 128:(dt + 1) * 128],
                                start=(dt == 0),
                                stop=(dt == DT - 1 and not has_qkv_bias))
                        if has_qkv_bias:
                            nc.tensor.matmul(ps_v[:, sl], ones_128[:],
                                             bv_sb[:], start=False,
                                             stop=True)
                        if last:
                            kt0 = Q0 // 128
                            nc.vector.tensor_copy(
                                v4[b][:, kt0:kt0 + 4, :, 0:64],
                                ps_v[:].rearrange("p (tt h j) -> p tt h j",
                                                  tt=4, h=2))
                    return emit

                groups = [qk_half(0, 0), qk_half(0, 1),
                          qk_half(1, 0), qk_half(1, 1)]
                groups += [v_block(tt, tt == 3) for tt in range(4)]
                return st, groups

            def proj_groups(st, ot_full):
                """Projection of a finished chunk, split into 4 groups of
                2 matmuls + copies, ending with the single store DMA."""
                ch = st["b"] * NCHB + st["qc"]
                ysb = yp.tile([128, CHW], bf16, tag="y", name="ysb")

                def pair(g):
                    def emit():
                        for oi in (2 * g, 2 * g + 1):
                            ps_y = ps_m_p.tile([128, 512], f32, tag="m",
                                               name="ps_y")
                            nc.tensor.matmul(
                                ps_y[:], pw_sb[:, oi * 128:(oi + 1) * 128],
                                ot_full[:], start=True, stop=True)
                            nc.vector.tensor_copy(
                                ysb[:, oi * 512:(oi + 1) * 512], ps_y[:])
                        if g == 3:
                            nc.sync.dma_start(
                                yTt[:, ch * CHW:(ch + 1) * CHW], ysb[:])
                    return emit
                return [pair(g) for g in range(4)]

            def attention(st, fillers):
                """Attention rounds of chunk st, with filler PE groups (next
                chunk's qkv + prev chunk's proj) interleaved so the PE never
                starves while ACT streams the exps."""
                b, qc = st["b"], st["qc"]
                Q0 = qc * QCH
                QT_cur = st["QT"]
                kt0 = Q0 // 128
                nkb = kt0 + 4
                O = [ps_o_p.tile([65, 512], f32, tag="o", name=f"O{h}")
                     for h in range(2)]
                # diag blocks first (gives affine_select extra slack),
                # then past blocks; S one kb ahead of O hides exp latency
                kb_order = list(range(kt0, nkb)) + list(range(0, kt0))
                pts = {}

                def emit_S(kb):
                    qstart = max(0, 128 * kb - Q0)
                    # both heads' scores in one 2-bank psum tile; K=128
                    # zero-padded KT keeps the PE at 1 cycle/row, and a
                    # single exp over [p, 2, q] halves ACT op overhead
                    ps_sc = ps_s_p.tile([128, 2 * QCH], f32, tag="s",
                                        name="ps_sc")
                    for h in range(2):
                        nc.tensor.matmul(
                            ps_sc[:, QCH * h + qstart:QCH * h + QCH],
                            kt2[b][:, h, kb * 128:(kb + 1) * 128],
                            QT_cur[:, qstart:QCH],
                            start=True, stop=True)
                    pt = ptp.tile([128, 2 * QCH], bf16, tag="pt", name="pt")
                    sc4 = ps_sc[:].rearrange("p (h q) -> p h q", h=2)
                    pt4 = pt[:].rearrange("p (h q) -> p h q", h=2)
                    nc.scalar.activation(pt4[:, :, qstart:QCH],
                                         sc4[:, :, qstart:QCH],
                                         AF.Exp, bias=0.0, scale=0.125)
                    if 128 * kb >= Q0:
                        ds = 128 * kb - Q0
                        for h in range(2):
                            nc.gpsimd.affine_select(
                                out=pt[:, QCH * h + ds:QCH * h + ds + 128],
                                in_=pt[:, QCH * h + ds:QCH * h + ds + 128],
                                pattern=[[1, 128]], compare_op=AL.is_ge,
                                fill=0.0, base=0, channel_multiplier=-1)
                    pts[kb] = pt

                def emit_O(kb, first, last):
                    qstart = max(0, 128 * kb - Q0)
                    pt = pts.pop(kb)
                    for h in range(2):
                        nc.tensor.matmul(
                            O[h][:, qstart:QCH],
                            v4[b][:, kb, h, :],
                            pt[:, QCH * h + qstart:QCH * h + QCH],
                            start=first, stop=last)

                done = 0
                for i, kb in enumerate(kb_order):
                    emit_S(kb)
                    target = (i + 1) * len(fillers) // nkb
                    while done < target:
                        fillers[done]()
                        done += 1
                    if i >= 1:
                        emit_O(kb_order[i - 1], first=(i == 1), last=False)
                emit_O(kb_order[-1], first=(len(kb_order) == 1), last=True)
                while done < len(fillers):
                    fillers[done]()
                    done += 1
                st["O"] = O

            # ---- software pipeline: A(ci) interleaved with X(ci+1)+P(ci-1)
            xa_t = {0: load_chunk(0)}
            if NCH_ALL > 1:
                xa_t[1] = load_chunk(1)
            for KT in kt_zero_dmas:
                nc.sync.dma_start(KT[:], ktz[:])
            st_cur, groups0 = x_stage_groups(0, xa_t[0])
            for g in groups0:       # prologue: X(0), nothing to overlap yet
                g()
            prev_st = None
            for ci in range(NCH_ALL):
                if ci + 2 < NCH_ALL:
                    xa_t[ci + 2] = load_chunk(ci + 2)
                if ci + 1 < NCH_ALL:
                    st_next, xg = x_stage_groups(ci + 1, xa_t[ci + 1])
                else:
                    st_next, xg = None, []
                if prev_st is not None:
                    pg = proj_groups(prev_st, norm_part(prev_st))
                else:
                    pg = []
                # proj last: gives the norm chain (ln/exp/bcast/mult of the
                # previous chunk) time to complete before its first consumer
                fillers = xg + pg
                attention(st_cur, fillers)
                prev_st = st_cur
                st_cur = st_next
            for g in proj_groups(prev_st, norm_part(prev_st)):
                g()
    nc.compile()
    return nc


# ---------------------------------------------------------------- host side

def _rope_tables(T):
    """cos / sign-baked-sin tables in the PERM64 feature order, bf16."""
    inv_freq = 1.0 / (ROPE_BASE ** (np.arange(0, HD, 2, dtype=np.float64)
                                    / HD))
    pos = np.arange(T, dtype=np.float64)
    ang = np.outer(pos, inv_freq)          # [T, 32]
    cos = np.cos(ang)                      # [T, 32]
    sin = np.sin(ang)
    p = np.arange(128)
    d = np.asarray(PERM64)[p % 64]         # original dim held by partition p
    cos_t = cos[:, d % 32].T               # [128, T]
    sgn = np.where(d < 32, -1.0, 1.0)
    sinm_t = (sin[:, d % 32] * sgn[None, :]).T
    return (np.ascontiguousarray(cos_t).astype(BF16),
            np.ascontiguousarray(sinm_t).astype(BF16))


def make_core_inputs(x, qkv_w, qkv_b, proj_w, NB, T, has_qkv_bias):
    NTOK = NB * T
    NCH = NTOK // QCH
    xf = np.asarray(x).reshape(NTOK, HIDDEN).astype(BF16)
    # xTt[p, ch, dt, c] = x[ch*512+c, dt*128+p]
    xTt = np.ascontiguousarray(
        xf.reshape(NCH, QCH, DT, 128).transpose(3, 0, 2, 1)
    ).reshape(128, NCH * CHW)
    cos_t, sinm_t = _rope_tables(T)
    ktz = np.zeros((128, 2 * T), dtype=BF16)
    qkv_w = np.asarray(qkv_w)
    qkv_b = np.asarray(qkv_b)
    perm = np.asarray(PERM64)
    in_maps = []
    for c in range(N_CORES):
        col = HD * H2 * c
        # q/k columns in PERM64 order (per head), v natural
        qcols = np.concatenate([col + perm, col + 64 + perm])
        wq = qkv_w[:, qcols]
        wk = qkv_w[:, HIDDEN + qcols]
        wv = qkv_w[:, 2 * HIDDEN + col:2 * HIDDEN + col + 128]
        wqk_t = np.zeros((128, DT * 256), dtype=np.float32)
        wv_t = np.zeros((128, DT * 128), dtype=np.float32)
        for dt in range(DT):
            rows = slice(dt * 128, (dt + 1) * 128)
            wqk_t[:, dt * 256:dt * 256 + 128] = wq[rows]
            wqk_t[:, dt * 256 + 128:dt * 256 + 256] = wk[rows]
            wv_t[:, dt * 128:(dt + 1) * 128] = wv[rows]
        pwc = np.ascontiguousarray(
            proj_w[col:col + 128, :]).astype(BF16)
        m = {
            "xTt": xTt, "wqk": wqk_t.astype(BF16), "wv": wv_t.astype(BF16),
            "pw": pwc, "cos_t": cos_t, "sinm_t": sinm_t, "ktz": ktz,
        }
        if has_qkv_bias:
            m["bqk"] = np.concatenate(
                [qkv_b[qcols], qkv_b[HIDDEN + qcols]])[None, :].astype(BF16)
            m["bv"] = qkv_b[2 * HIDDEN + col:
                            2 * HIDDEN + col + 128][None, :].astype(BF16)
        in_maps.append(m)
    return in_maps


_PROGRAM_CACHE = {}


def _get_program(NB, T, has_qkv_bias):
    key = (NB, T, has_qkv_bias)
    if key not in _PROGRAM_CACHE:
        _PROGRAM_CACHE[key] = build_program(NB, T, has_qkv_bias)
    return _PROGRAM_CACHE[key]


def run(x, qkv_w, qkv_b, proj_w, proj_b, NB, T, trace=False):
    has_qkv_bias = bool(np.any(np.asarray(qkv_b)))
    nc = _get_program(NB, T, has_qkv_bias)
    in_maps = make_core_inputs(x, qkv_w, qkv_b, proj_w, NB, T, has_qkv_bias)
    res = bass_utils.run_bass_kernel_spmd(
        nc, in_maps, core_ids=list(range(N_CORES)), trace=trace)
    acc = res.results[0]["yTt"].astype(np.float32)
    for c in range(1, N_CORES):
        acc += res.results[c]["yTt"].astype(np.float32)
    NTOK = NB * T
    NCH = NTOK // QCH
    # y[ch*512+c, oi*128+p] = acc[p, ch, oi, c]
    y = acc.reshape(128, NCH, DT, QCH).transpose(1, 3, 2, 0).reshape(
        NTOK, HIDDEN)
    out = (y.reshape(NB, T, HIDDEN)
           + np.asarray(proj_b)[None, None, :].astype(np.float32))
    return out, res


def kernel(x, qkv_w, qkv_b, proj_w, proj_b):
    x = np.asarray(x)
    B, L, D = x.shape
    out, _ = run(x, np.asarray(qkv_w), np.asarray(qkv_b),
                 np.asarray(proj_w), np.asarray(proj_b), NB=B, T=L)
    return out.astype(np.float32)
